# revision 1
# baseline (speedup 1.0000x reference)
"""BiLSTM-CRF decode kernel for Trainium2 (8 NeuronCores, batch-sharded).

Model: embedding lookup -> 2-layer BiLSTM (H=128/dir) -> linear -> CRF Viterbi.
Output: [B, T] int32 best-path tags.

Sharding: data-parallel over batch, B=128 -> 16 rows per core. Everything on
one core is laid out feature-major ([feature partitions, batch free]) so the
serial time recurrences run on full-width engine ops.
"""

import os
import numpy as np

import concourse.bass as bass
import concourse.bacc as bacc
import concourse.tile as tile
import concourse.mybir as mybir
from concourse.bass import IndirectOffsetOnAxis
from concourse import bass_utils

B, TFULL, V, D, H, K = 128, 512, 50000, 128, 128, 32
NCORES = 8
BL = B // NCORES  # 16 batch rows per core

f32 = mybir.dt.float32
bf16 = mybir.dt.bfloat16
i32 = mybir.dt.int32
u32 = mybir.dt.uint32
AF = mybir.ActivationFunctionType
ALU = mybir.AluOpType
AX = mybir.AxisListType

# torch gate order is [i, f, g, o]; we use [i, f, o, g] so the sigmoid gates
# (i, f, o) are contiguous and tanh(g) is the last chunk.
_PERM = np.r_[0:H, H:2 * H, 3 * H:4 * H, 2 * H:3 * H]

LAST_RESULTS = None  # BassKernelResults of the most recent run (for test.py)


def _f(x):
    return np.ascontiguousarray(np.asarray(x, dtype=np.float32))


def _host_consts(emb, w_ih_l0, w_hh_l0, b_l0, w_ih_l1, w_hh_l1, b_l1,
                 W_out, b_out, start_t, end_t, trans):
    """Build all per-core-identical device input arrays."""
    c = {}
    c["embt"] = _f(emb)

    for d in (0, 1):
        c[f"wx0{d}"] = _f(np.asarray(w_ih_l0)[d][_PERM].T)      # [128, 512]
        c[f"wh0{d}"] = _f(np.asarray(w_hh_l0)[d][_PERM].T)      # [128, 512]
        w1 = np.asarray(w_ih_l1)[d][_PERM]                       # [512, 256]
        c[f"wxA1{d}"] = _f(w1[:, :H].T)                          # [128, 512]
        c[f"wxB1{d}"] = _f(w1[:, H:].T)                          # [128, 512]
        c[f"wh1{d}"] = _f(np.asarray(w_hh_l1)[d][_PERM].T)       # [128, 512]

    for l, bl in ((0, b_l0), (1, b_l1)):
        bk = np.zeros((8, 128), dtype=np.float32)
        for d in (0, 1):
            bperm = np.asarray(bl)[d][_PERM]
            for ci in range(4):
                bk[d * 4 + ci, :] = bperm[ci * 128:(ci + 1) * 128]
        c[f"biasK{l}"] = _f(bk)
    ind = np.zeros((8, 128), dtype=np.float32)
    for d in (0, 1):
        for ci in range(4):
            ind[d * 4 + ci, d * 64 + ci * 16: d * 64 + (ci + 1) * 16] = 1.0
    c["chunkInd8"] = _f(ind)

    # Viterbi score columns are stored jl-major: column i' holds tag
    # pi[i'] = (i' % 4) * 8 + i' // 4 (so each pick-matmul writes a
    # contiguous 4-column block).
    pi = (np.arange(32) % 4) * 8 + np.arange(32) // 4

    WoT = _f(np.asarray(W_out).T)                                # [256, 32]
    c["woutA"] = _f(WoT[:H][:, pi])
    c["woutB"] = _f(WoT[H:][:, pi])
    c["bo1"] = _f(np.asarray(b_out)[pi][None, :])                # [1, 32]
    c["ones1"] = np.ones((1, 128), dtype=np.float32)

    km = np.arange(128)
    rep_full = (km[:, None] % 16 == km[None, :] % 16).astype(np.float32)
    for q in range(8):
        bm = rep_full.copy()
        bm[(km // 16) != q, :] = 0.0
        c[f"band{q}"] = _f(bm)

    jlv = np.arange(128) // 16                                   # [128]
    gv = 8 * np.arange(4)                                        # [4]
    trans_np = _f(trans)
    # transP[p, g*32 + i'] = trans[pi[i'], g*8 + jl(p)]
    c["transP"] = _f(trans_np[pi].T[jlv[:, None] + gv[None, :]]
                     .reshape(128, 128))
    c["iotaI"] = _f(np.tile(pi.astype(np.float32), (128, 4)))
    c["iotaF"] = _f(np.tile(pi.astype(np.float32), (BL, 1)))     # [16, 32]

    jp = np.empty(32, dtype=np.float32)
    for jl in range(8):
        for g in range(4):
            jp[jl * 4 + g] = g * 8 + jl
    c["iotaJP"] = _f(np.tile(jp, (BL, 1)))                       # [16, 32]

    c["ms0"] = _f(np.asarray(start_t)[jlv[:, None] + gv[None, :]])  # [128, 4]
    c["end128"] = _f(np.tile(np.asarray(end_t)[pi], (128, 1)))   # [128, 32]
    c["ident"] = np.eye(128, dtype=np.float32)
    return c


def _ids_for_core(inputs_np, core, T_):
    ids_c = inputs_np[core * BL:(core + 1) * BL, :T_]            # [16, T]
    flat = np.ascontiguousarray(ids_c.T).reshape(-1)             # t-major
    nblk = (BL * T_) // 128
    return np.ascontiguousarray(flat.reshape(nblk, 128).T.astype(np.int32))


def _build_program(T_):
    """Build the full single-core Bass program (identical across cores)."""
    TOK = BL * T_
    NBLK = TOK // 128

    nc = bacc.Bacc()
    d = {}

    def din(name, shape, dtype=f32):
        d[name] = nc.dram_tensor(name, list(shape), dtype, kind="ExternalInput")
        return d[name]

    din("ids_p", [128, NBLK], i32)
    din("embt", [V, D])
    for dd in (0, 1):
        din(f"wx0{dd}", [128, 512])
        din(f"wh0{dd}", [128, 512])
        din(f"wxA1{dd}", [128, 512])
        din(f"wxB1{dd}", [128, 512])
        din(f"wh1{dd}", [128, 512])
    din("biasK0", [8, 128])
    din("biasK1", [8, 128])
    din("chunkInd8", [8, 128])
    din("woutA", [128, K])
    din("woutB", [128, K])
    din("bo1", [1, K])
    din("ones1", [1, 128])
    for q in range(8):
        din(f"band{q}", [128, 128])
    din("transP", [128, 128])
    din("iotaI", [128, 128])
    din("iotaJP", [BL, K])
    din("iotaF", [BL, K])
    din("ms0", [128, 4])
    din("end128", [128, K])
    din("ident", [128, 128])
    out_ids = nc.dram_tensor("out_ids", [BL, T_], i32, kind="ExternalOutput")

    with tile.TileContext(nc) as tc:
        _emit(nc, tc, d, out_ids, T_, TOK, NBLK)
    nc.compile()
    return nc


def _lstm_layer(nc, wk, zpool, T_, xparts_f, xparts_b, wh_f, wh_b,
                biasK_sb, chunkInd_sb, hF, hB):
    """One BiLSTM layer; fwd and bwd directions interleaved per step.

    xparts_*: list of (lhsT_sbuf [128,512], rhs_fn(t) -> AP[128,16]) K-parts.
    hF/hB: [128, 16*T_] output buffers (also read for the recurrent matmul).
    """
    cF = wk.tile([128, 16], f32, tag="cF")
    cB = wk.tile([128, 16], f32, tag="cB")
    cs = (cF, cB)

    for s in range(T_):
        tf, tb = s, T_ - 1 - s
        zp = zpool.tile([128, 128], f32, tag="z")
        nc.tensor.matmul(out=zp[:], lhsT=biasK_sb[:], rhs=chunkInd_sb[:],
                         start=True, stop=False, skip_group_check=True)
        for dd, t, xparts, wh, hprev in (
            (0, tf, xparts_f, wh_f, hF),
            (1, tb, xparts_b, wh_b, hB),
        ):
            for ci in range(4):
                oap = zp[:, dd * 64 + ci * 16: dd * 64 + (ci + 1) * 16]
                mms = [(wT[:, ci * 128:(ci + 1) * 128], rhs_fn(t))
                       for (wT, rhs_fn) in xparts]
                if s > 0:
                    tp = t - 1 if dd == 0 else t + 1
                    mms.append((wh[:, ci * 128:(ci + 1) * 128],
                                hprev[:, 16 * tp:16 * tp + 16]))
                for j, (lh, rh) in enumerate(mms):
                    nc.tensor.matmul(out=oap, lhsT=lh, rhs=rh,
                                     start=False, stop=(j == len(mms) - 1),
                                     skip_group_check=True)

        # fwd and bwd get fully separate ACT/DVE chains so the scheduler can
        # pipeline one direction's gates against the other's matmuls.
        zp3 = zp.rearrange("p (d c2) -> p d c2", d=2)
        for dd, t, hout in ((0, tf, hF), (1, tb, hB)):
            c_d = cs[dd]
            sig = wk.tile([128, 48], f32, tag=f"sig{dd}",
                          name=f"sig{dd}_{s}")
            nc.scalar.activation(out=sig[:], in_=zp3[:, dd, 0:48],
                                 func=AF.Sigmoid)
            tg = wk.tile([128, 16], f32, tag=f"tg{dd}", name=f"tg{dd}_{s}")
            nc.scalar.activation(out=tg[:], in_=zp3[:, dd, 48:64],
                                 func=AF.Tanh)
            if s == 0:
                nc.vector.tensor_mul(out=c_d[:], in0=sig[:, 0:16], in1=tg[:])
            else:
                t1 = wk.tile([128, 16], f32, tag=f"t1{dd}",
                             name=f"t1{dd}_{s}")
                nc.vector.tensor_mul(out=t1[:], in0=sig[:, 0:16], in1=tg[:])
                t2 = wk.tile([128, 16], f32, tag=f"t2{dd}",
                             name=f"t2{dd}_{s}")
                nc.vector.tensor_mul(out=t2[:], in0=sig[:, 16:32],
                                     in1=c_d[:])
                nc.vector.tensor_add(out=c_d[:], in0=t1[:], in1=t2[:])
            tct = wk.tile([128, 16], f32, tag=f"tct{dd}",
                          name=f"tct{dd}_{s}")
            nc.scalar.activation(out=tct[:], in_=c_d[:], func=AF.Tanh)
            nc.vector.tensor_mul(out=hout[:, 16 * t:16 * t + 16],
                                 in0=sig[:, 32:48], in1=tct[:])


def _emit(nc, tc, d, out_ids, T_, TOK, NBLK):
    # bisection aid: stop after a given stage (embed, l0, l1, proj, vit, full)
    stage = os.environ.get("KERNEL_STAGE", "full")
    order = ["embed", "l0", "l1", "proj", "vit", "full"]
    lvl = order.index(stage)
    with tc.tile_pool(name="gc", bufs=1) as gc:
        band_sb = [gc.tile_from(d[f"band{q}"][:], name=f"band{q}sb")
                   for q in range(8)]
        transP_sb = gc.tile_from(d["transP"][:])
        iotaI_sb = gc.tile_from(d["iotaI"][:])
        iotaJP_sb = gc.tile_from(d["iotaJP"][:])
        iotaF_sb = gc.tile_from(d["iotaF"][:])
        ms0_sb = gc.tile_from(d["ms0"][:])
        end_sb = gc.tile_from(d["end128"][:])
        ident_sb = gc.tile_from(d["ident"][:])
        ids_sb = gc.tile_from(d["ids_p"][:])
        woutA_sb = gc.tile_from(d["woutA"][:])
        woutB_sb = gc.tile_from(d["woutB"][:])
        bo1_sb = gc.tile_from(d["bo1"][:])
        ones1_sb = gc.tile_from(d["ones1"][:])

        outT = gc.tile([BL, T_], f32)
        em2 = gc.tile([128, NBLK * K], f32)
        histAll = gc.tile([128, 4 * T_], f32)
        out_sb = gc.tile([BL, T_], i32)

        transP3 = transP_sb.rearrange("p (g i) -> p g i", g=4)
        iotaI3 = iotaI_sb.rearrange("p (g i) -> p g i", g=4)

        with tc.tile_pool(name="h1p", bufs=1) as h1p:
            h1F = h1p.tile([128, TOK], f32)
            h1B = h1p.tile([128, TOK], f32)

            # ---------------- embedding gather + layer 0 ----------------
            with tc.tile_pool(name="px", bufs=1) as px, \
                 tc.tile_pool(name="ge", bufs=4) as ge, \
                 tc.tile_pool(name="pe", bufs=2, space="PSUM") as pe, \
                 tc.tile_pool(name="zp0", bufs=2, space="PSUM") as zp0, \
                 tc.tile_pool(name="wk0", bufs=3) as wk0:
                xT = px.tile([128, TOK], f32)
                w0 = {dd: (px.tile_from(d[f"wx0{dd}"][:], name=f"wx0{dd}sb"),
                           px.tile_from(d[f"wh0{dd}"][:], name=f"wh0{dd}sb"))
                      for dd in (0, 1)}
                biasK0_sb = px.tile_from(d["biasK0"][:])
                chunkInd_sb = px.tile_from(d["chunkInd8"][:])

                # gather order: both ends first so fwd/bwd can start early
                order = []
                for k in range((NBLK + 1) // 2):
                    order.append(k)
                    if NBLK - 1 - k != k:
                        order.append(NBLK - 1 - k)
                for n, k in enumerate(order):
                    g_t = ge.tile([128, 128], f32, tag="g")
                    nc.gpsimd.indirect_dma_start(
                        out=g_t[:], out_offset=None, in_=d["embt"][:],
                        in_offset=IndirectOffsetOnAxis(
                            ap=ids_sb[:, k:k + 1], axis=0))
                    tp = pe.tile([128, 128], f32, tag="tp")
                    nc.tensor.transpose(tp[:], g_t[:], ident_sb[:])
                    dst = xT[:, 128 * k:128 * (k + 1)]
                    if n % 2 == 0:
                        nc.vector.tensor_copy(out=dst, in_=tp[:])
                    else:
                        nc.scalar.copy(out=dst, in_=tp[:])

                xf = [(w0[0][0], lambda t: xT[:, 16 * t:16 * t + 16])]
                xb = [(w0[1][0], lambda t: xT[:, 16 * t:16 * t + 16])]
                if lvl >= 1:
                    _lstm_layer(nc, wk0, zp0, T_, xf, xb, w0[0][1], w0[1][1],
                                biasK0_sb, chunkInd_sb, h1F, h1B)

            # ---------------- layer 1 + emission projection ----------------
            with tc.tile_pool(name="pw1", bufs=1) as pw1:
                w1 = {dd: (pw1.tile_from(d[f"wxA1{dd}"][:], name=f"wxA1{dd}sb"),
                           pw1.tile_from(d[f"wxB1{dd}"][:], name=f"wxB1{dd}sb"),
                           pw1.tile_from(d[f"wh1{dd}"][:], name=f"wh1{dd}sb"))
                      for dd in (0, 1)}
                biasK1_sb = pw1.tile_from(d["biasK1"][:])
                chunkInd1_sb = pw1.tile_from(d["chunkInd8"][:])

                with tc.tile_pool(name="ph2", bufs=1) as ph2, \
                     tc.tile_pool(name="zp1", bufs=2, space="PSUM") as zp1, \
                     tc.tile_pool(name="wk1", bufs=3) as wk1, \
                     tc.tile_pool(name="pj", bufs=2, space="PSUM") as pj:
                    h2F = ph2.tile([128, TOK], f32)
                    h2B = ph2.tile([128, TOK], f32)

                    xf = [(w1[0][0], lambda t: h1F[:, 16 * t:16 * t + 16]),
                          (w1[0][1], lambda t: h1B[:, 16 * t:16 * t + 16])]
                    xb = [(w1[1][0], lambda t: h1F[:, 16 * t:16 * t + 16]),
                          (w1[1][1], lambda t: h1B[:, 16 * t:16 * t + 16])]
                    if lvl >= 2:
                        _lstm_layer(nc, wk1, zp1, T_, xf, xb, w1[0][2],
                                    w1[1][2], biasK1_sb, chunkInd1_sb,
                                    h2F, h2B)

                    # emissions: em2[(t%8)*16+b, (t//8)*32+j]
                    for k in range(NBLK if lvl >= 3 else 0):
                        ep = pj.tile([128, K], f32, tag="ep")
                        nc.tensor.matmul(out=ep[:], lhsT=ones1_sb[:],
                                         rhs=bo1_sb[:], start=True, stop=False,
                                         skip_group_check=True)
                        nc.tensor.matmul(out=ep[:],
                                         lhsT=h2F[:, 128 * k:128 * (k + 1)],
                                         rhs=woutA_sb[:], start=False,
                                         stop=False, skip_group_check=True)
                        nc.tensor.matmul(out=ep[:],
                                         lhsT=h2B[:, 128 * k:128 * (k + 1)],
                                         rhs=woutB_sb[:], start=False,
                                         stop=True, skip_group_check=True)
                        dst = em2[:, K * k:K * (k + 1)]
                        if k % 2 == 0:
                            nc.vector.tensor_copy(out=dst, in_=ep[:])
                        else:
                            nc.scalar.copy(out=dst, in_=ep[:])

        # ---------------- Viterbi forward ----------------
        hA3 = histAll.rearrange("p (g t) -> p g t", t=T_)
        with tc.tile_pool(name="srp", bufs=2, space="PSUM") as srp, \
             tc.tile_pool(name="vt", bufs=3) as vt:
            nc.vector.memset(hA3[:, :, 0], 0.0)
            prev_ms = ms0_sb[:]
            for s in range(1, (T_ + 1) if lvl >= 4 else 0):
                sr = srp.tile([128, K], f32, tag="sr")
                srg = sr.rearrange("p (jl g) -> p jl g", g=4)
                tl = (s - 1) % 8
                blk = (s - 1) // 8
                nc.tensor.matmul(out=sr[:],
                                 lhsT=band_sb[tl][:],
                                 rhs=em2[:, K * blk:K * (blk + 1)],
                                 start=True, stop=False,
                                 skip_group_check=True)
                for jl in range(8):
                    nc.tensor.matmul(out=srg[:, jl, :],
                                     lhsT=band_sb[jl][:],
                                     rhs=prev_ms,
                                     start=False,
                                     stop=(jl == 7 and s != T_),
                                     skip_group_check=True)
                if s == T_:
                    # fold end transition scores into the final step
                    nc.tensor.matmul(out=sr[:], lhsT=band_sb[0][:],
                                     rhs=end_sb[:], start=False,
                                     stop=True, skip_group_check=True)
                    sf = vt.tile([BL, K], f32, tag="sf")
                    nc.vector.tensor_copy(out=sf[:], in_=sr[0:16, :])
                    mfin = vt.tile([BL, 1], f32, tag="mfin")
                    nc.vector.reduce_max(out=mfin[:], in_=sf[:], axis=AX.X)
                    eqf = vt.tile([BL, K], f32, tag="eqf")
                    nc.vector.tensor_tensor(
                        out=eqf[:], in0=sf[:],
                        in1=mfin[:].to_broadcast([BL, K]), op=ALU.is_equal)
                    eif = vt.tile([BL, K], f32, tag="eif")
                    nc.vector.tensor_mul(out=eif[:], in0=eqf[:],
                                         in1=iotaF_sb[:])
                    nc.vector.reduce_max(out=outT[:, T_ - 1:T_], in_=eif[:],
                                         axis=AX.X)
                    break

                cand = vt.tile([128, 128], f32, tag="cand")
                cand3 = cand.rearrange("p (g i) -> p g i", g=4)
                srb = sr[:].rearrange("p (o i) -> p o i", o=1) \
                           .to_broadcast([128, 4, K])
                nc.vector.tensor_add(out=cand3, in0=srb, in1=transP3)
                ms = vt.tile([128, 4], f32, tag="ms")
                nc.vector.reduce_max(out=ms[:], in_=cand3, axis=AX.X)
                eqv = vt.tile([128, 128], f32, tag="eqv")
                eq3 = eqv.rearrange("p (g i) -> p g i", g=4)
                msb = ms[:].rearrange("p (g o) -> p g o", o=1) \
                           .to_broadcast([128, 4, K])
                nc.vector.tensor_tensor(out=eq3, in0=cand3, in1=msb,
                                        op=ALU.is_equal)
                eiv = vt.tile([128, 128], f32, tag="eiv")
                ei3 = eiv.rearrange("p (g i) -> p g i", g=4)
                nc.vector.tensor_mul(out=ei3, in0=eq3, in1=iotaI3)
                nc.vector.reduce_max(out=hA3[:, :, s], in_=ei3, axis=AX.X)
                prev_ms = ms[:]

        # ---------------- backtrace ----------------
        with tc.tile_pool(name="pbt", bufs=1) as pbt, \
             tc.tile_pool(name="bt", bufs=2) as bt:
            if lvl >= 5:
                histAllB = pbt.tile([128, 4 * T_], bf16)
                nc.vector.tensor_copy(out=histAllB[:], in_=histAll[:])
                histB = pbt.tile([BL, 32 * T_], bf16)
                hB4 = histB.rearrange("p (jl g t) -> p jl g t", jl=8, g=4)
                for jl in range(8):
                    src = histAllB[16 * jl:16 * (jl + 1), :] \
                        .rearrange("p (g t) -> p g t", t=T_)
                    nc.sync.dma_start(out=hB4[:, jl], in_=src)

                for s in range(T_ - 2, -1, -1):
                    oh = bt.tile([BL, K], bf16, tag="oh")
                    nc.vector.tensor_scalar(out=oh[:], in0=iotaJP_sb[:],
                                            scalar1=outT[:, s + 1:s + 2],
                                            scalar2=None, op0=ALU.is_equal)
                    oh3 = oh.rearrange("p (jl g) -> p jl g", jl=8)
                    scr = bt.tile([BL, K], bf16, tag="scr")
                    scr3 = scr.rearrange("p (jl g) -> p jl g", jl=8)
                    nc.vector.tensor_mul(out=scr3, in0=oh3,
                                         in1=hB4[:, :, :, s + 1])
                    nc.vector.reduce_max(out=outT[:, s:s + 1], in_=scr[:],
                                         axis=AX.X)

                nc.vector.tensor_copy(out=out_sb[:], in_=outT[:])
            else:
                nc.vector.memset(out_sb[:], 0)
            nc.sync.dma_start(out=out_ids[:], in_=out_sb[:])


def _run(inputs_np, consts, T_):
    global LAST_RESULTS
    nc = _build_program(T_)
    in_maps = []
    for core in range(NCORES):
        m = dict(consts)
        m["ids_p"] = _ids_for_core(inputs_np, core, T_)
        in_maps.append(m)
    trace = bool(int(os.environ.get("KERNEL_TRACE", "0")))
    res = bass_utils.run_bass_kernel_spmd(
        nc, in_maps, core_ids=list(range(NCORES)), trace=trace)
    LAST_RESULTS = res
    return np.concatenate([r["out_ids"] for r in res.results], axis=0)


def kernel(inputs, tags, emb, w_ih_l0, w_hh_l0, b_l0,
           w_ih_l1, w_hh_l1, b_l1, W_out, b_out,
           start_t, end_t, trans, _T=TFULL):
    del tags  # unused at decode time
    inputs_np = np.ascontiguousarray(np.asarray(inputs, dtype=np.int32))
    consts = _host_consts(emb, w_ih_l0, w_hh_l0, b_l0, w_ih_l1, w_hh_l1,
                          b_l1, W_out, b_out, start_t, end_t, trans)
    return _run(inputs_np, consts, _T)



# revision 7
# speedup vs baseline: 1.8459x; 1.8459x over previous
"""BiLSTM-CRF decode kernel for Trainium2 (8 NeuronCores, batch-sharded).

Model: embedding lookup -> 2-layer BiLSTM (H=128/dir) -> linear -> CRF Viterbi.
Output: [B, T] int32 best-path tags.

Sharding: data-parallel over batch, B=128 -> 16 rows per core. Feature-major
layout ([feature partitions, batch free]) so the serial time recurrences run
full-width engine ops.

Perf structure (v2):
- All LSTM/projection matmuls run in fp16 (1-pass on the PE; fp32 is 2-pass).
- The input-to-gates GEMMs (x @ Wih) are hoisted out of the recurrence into
  bulk N=256 matmuls that pre-accumulate zx+bias into PSUM half-banks; the
  in-loop recurrence only adds Whh @ h into the same PSUM cells. The gate
  nonlinearities read PSUM directly.
- Viterbi forward uses a replicated-score formulation: one masked DVE mul +
  one rep16 matmul replaces the 8 per-step band broadcasts.
- Output linear bias b_out is folded into trans/start_t host-side.
"""

import os
import numpy as np

import concourse.bass as bass
import concourse.bacc as bacc
import concourse.tile as tile
import concourse.mybir as mybir
from concourse.bass import IndirectOffsetOnAxis
from concourse import bass_utils

B, TFULL, V, D, H, K = 128, 512, 50000, 128, 128, 32
NCORES = 8
BL = B // NCORES  # 16 batch rows per core

f32 = mybir.dt.float32
f16 = mybir.dt.float16
bf16 = mybir.dt.bfloat16
i32 = mybir.dt.int32
AF = mybir.ActivationFunctionType
ALU = mybir.AluOpType
AX = mybir.AxisListType

# torch gate order is [i, f, g, o]; we use [i, f, o, g] so the sigmoid gates
# (i, f, o) are contiguous and tanh(g) is the last chunk.
_PERM = np.r_[0:H, H:2 * H, 3 * H:4 * H, 2 * H:3 * H]

LAST_RESULTS = None  # BassKernelResults of the most recent run (for test.py)


def _f(x):
    return np.ascontiguousarray(np.asarray(x, dtype=np.float32))


def _h(x):
    return np.ascontiguousarray(np.asarray(x, dtype=np.float16))


def _host_consts(emb, w_ih_l0, w_hh_l0, b_l0, w_ih_l1, w_hh_l1, b_l1,
                 W_out, b_out, start_t, end_t, trans):
    """Build all per-core-identical device input arrays."""
    c = {}
    c["embt"] = _f(emb)

    for d in (0, 1):
        c[f"wx0{d}"] = _f(np.asarray(w_ih_l0)[d][_PERM].T)      # [128, 512]
        c[f"wh0{d}"] = _f(np.asarray(w_hh_l0)[d][_PERM].T)      # [128, 512]
        w1 = np.asarray(w_ih_l1)[d][_PERM]                       # [512, 256]
        c[f"wxA1{d}"] = _f(w1[:, :H].T)                          # [128, 512]
        c[f"wxB1{d}"] = _f(w1[:, H:].T)                          # [128, 512]
        c[f"wh1{d}"] = _f(np.asarray(w_hh_l1)[d][_PERM].T)       # [128, 512]

    for l, bl in ((0, b_l0), (1, b_l1)):
        bk = np.zeros((1, 1024), dtype=np.float32)
        for d in (0, 1):
            bperm = np.asarray(bl)[d][_PERM]
            bk[0, d * 512:(d + 1) * 512] = bperm
        c[f"biasK{l}"] = np.ascontiguousarray(bk)
    c["ones16"] = np.ones((1, 512), dtype=np.float32)

    # Emission projection (b_out folded into trans/start below).
    WoT = _f(np.asarray(W_out).T)                                # [256, 32]
    c["woutA"] = _f(WoT[:H])
    c["woutB"] = _f(WoT[H:])

    km = np.arange(128)
    rep_full = (km[:, None] % 16 == km[None, :] % 16).astype(np.float32)
    c["rep16"] = _f(rep_full)
    for q in range(8):
        bm = rep_full.copy()
        bm[(km // 16) != q, :] = 0.0
        c[f"band{q}"] = _f(bm)

    # Viterbi, tag layout: score column i = tag i; per-partition-block jl
    # owns tags j = g*8 + jl (g in 0..3) stored as ms[:, g].
    trans_p = _f(trans) + _f(b_out)[None, :]                     # [32, 32]
    jlv = np.arange(128) // 16                                   # [128]
    gv = np.arange(4)
    # transP[(jl,b), g*32 + i] = trans_p[i, g*8 + jl]
    tp = np.zeros((128, 128), dtype=np.float32)
    for p in range(128):
        for g in range(4):
            tp[p, g * 32:(g + 1) * 32] = trans_p[:, g * 8 + jlv[p]]
    c["transP"] = tp
    # M8[(jl2,b), (g,jl)] = (jl2 == jl)
    m8 = np.zeros((128, 32), dtype=np.float32)
    for p in range(128):
        for g in range(4):
            for jl in range(8):
                m8[p, g * 8 + jl] = 1.0 if (p // 16) == jl else 0.0
    c["M8"] = m8
    c["iotaI"] = _f(np.tile(np.arange(32, dtype=np.float32), (128, 4)))
    c["startRow"] = _f(np.asarray(start_t) + np.asarray(b_out))[None, :]
    c["endRow"] = _f(np.asarray(end_t))[None, :]
    c["ones1"] = np.ones((1, 128), dtype=np.float32)
    c["iotaF"] = _f(np.tile(np.arange(32, dtype=np.float32), (BL, 1)))

    jp = np.empty(32, dtype=np.float32)
    for jl in range(8):
        for g in range(4):
            jp[jl * 4 + g] = g * 8 + jl
    c["iotaJP"] = _f(np.tile(jp, (BL, 1)))                       # [16, 32]

    c["ident16"] = np.eye(128, dtype=np.float32)
    return c


def _ids_for_core(inputs_np, core, T_):
    ids_c = inputs_np[core * BL:(core + 1) * BL, :T_]            # [16, T]
    flat = np.ascontiguousarray(ids_c.T).reshape(-1)             # t-major
    nblk = (BL * T_) // 128
    return np.ascontiguousarray(flat.reshape(nblk, 128).T.astype(np.int32))


def _build_program(T_):
    """Build the full single-core Bass program (identical across cores)."""
    TOK = BL * T_
    NBLK = TOK // 128

    nc = bacc.Bacc()
    d = {}

    def din(name, shape, dtype=f32):
        d[name] = nc.dram_tensor(name, list(shape), dtype, kind="ExternalInput")
        return d[name]

    din("ids_p", [128, NBLK], i32)
    din("embt", [V, D])
    for dd in (0, 1):
        din(f"wx0{dd}", [128, 512])
        din(f"wh0{dd}", [128, 512])
        din(f"wxA1{dd}", [128, 512])
        din(f"wxB1{dd}", [128, 512])
        din(f"wh1{dd}", [128, 512])
    din("biasK0", [1, 1024])
    din("biasK1", [1, 1024])
    din("ones16", [1, 512])
    din("woutA", [128, K])
    din("woutB", [128, K])
    din("rep16", [128, 128])
    for q in range(8):
        din(f"band{q}", [128, 128])
    din("transP", [128, 128])
    din("M8", [128, K])
    din("iotaI", [128, 128])
    din("startRow", [1, K])
    din("endRow", [1, K])
    din("ones1", [1, 128])
    din("iotaF", [BL, K])
    din("iotaJP", [BL, K])
    din("ident16", [128, 128])
    out_ids = nc.dram_tensor("out_ids", [BL, T_], i32, kind="ExternalOutput")

    with tile.TileContext(nc) as tc:
        _emit(nc, tc, d, out_ids, T_, TOK, NBLK)
    nc.compile()
    return nc


def _lstm_layer(nc, wk, zz, T_, xparts, wh_f, wh_b, biasK_sb, ones16_sb,
                hF, hB):
    """One BiLSTM layer; fwd and bwd directions interleaved per step.

    zz: PSUM tile [128, 4096] = 8 banks; bank (dd*4+ci) holds
    [buf(2), tb(16), b(16)] columns. Bulk zx+bias matmuls pre-fill 16-step
    half-banks; the in-loop Whh matmuls accumulate into single-step cells.
    xparts: per-dir list of (wxT [128,512] f16 sbuf, rhs [128, TOK] f16 sbuf).
    hF/hB: [128, TOK] f16 output buffers (also the recurrent matmul input).
    """
    NBLKS = T_ // 16
    zz6 = zz.rearrange("p (dd ci buf tb b) -> p dd ci buf tb b",
                       dd=2, ci=4, buf=2, tb=16)

    def off(dd, ci, buf):
        return ((dd * 4 + ci) * 2 + buf) * 256

    def bulk(dd, blk):
        buf = blk % 2
        for ci in range(4):
            o = zz[:, off(dd, ci, buf):off(dd, ci, buf) + 256]
            bsl = biasK_sb[:, (dd * 4 + ci) * 128:(dd * 4 + ci + 1) * 128]
            nc.tensor.matmul(out=o, lhsT=bsl,
                             rhs=ones16_sb[:, 0:256], start=True, stop=False,
                             skip_group_check=True)
            for (wT, rhs) in xparts[dd]:
                nc.tensor.matmul(out=o, lhsT=wT[:, ci * 128:(ci + 1) * 128],
                                 rhs=rhs[:, 256 * blk:256 * (blk + 1)],
                                 start=False, stop=False,
                                 skip_group_check=True)

    bulk(0, 0)
    bulk(1, NBLKS - 1)

    cF = wk.tile([128, 16], f32, tag="cF")
    cB = wk.tile([128, 16], f32, tag="cB")
    cs = (cF, cB)
    whs = (wh_f, wh_b)
    hs = (hF, hB)

    for s in range(T_):
        if s % 16 == 0 and s > 0:
            k = s // 16
            if k < NBLKS:
                bulk(0, k)
                bulk(1, NBLKS - 1 - k)
        tf, tb_t = s, T_ - 1 - s
        for dd, t in ((0, tf), (1, tb_t)):
            if s > 0:
                tp = t - 1 if dd == 0 else t + 1
                buf, tbi = (t // 16) % 2, t % 16
                wh = whs[dd]
                for ci in range(4):
                    o = zz[:, off(dd, ci, buf) + tbi * 16:
                           off(dd, ci, buf) + tbi * 16 + 16]
                    nc.tensor.matmul(out=o,
                                     lhsT=wh[:, ci * 128:(ci + 1) * 128],
                                     rhs=hs[dd][:, 16 * tp:16 * tp + 16],
                                     start=False, stop=True,
                                     skip_group_check=True)

        # fwd and bwd get fully separate ACT/DVE chains so the scheduler can
        # pipeline one direction's gates against the other's matmuls.
        for dd, t in ((0, tf), (1, tb_t)):
            buf, tbi = (t // 16) % 2, t % 16
            c_d = cs[dd]
            sig = wk.tile([128, 48], f32, tag=f"sig{dd}",
                          name=f"sig{dd}_{s}")
            nc.scalar.activation(out=sig[:], in_=zz6[:, dd, 0:3, buf, tbi],
                                 func=AF.Sigmoid)
            tg = wk.tile([128, 16], f32, tag=f"tg{dd}", name=f"tg{dd}_{s}")
            nc.scalar.activation(out=tg[:], in_=zz6[:, dd, 3, buf, tbi],
                                 func=AF.Tanh)
            if s == 0:
                nc.vector.tensor_mul(out=c_d[:], in0=sig[:, 0:16], in1=tg[:])
            else:
                t1 = wk.tile([128, 16], f32, tag=f"t1{dd}",
                             name=f"t1{dd}_{s}")
                nc.vector.tensor_mul(out=t1[:], in0=sig[:, 0:16], in1=tg[:])
                t2 = wk.tile([128, 16], f32, tag=f"t2{dd}",
                             name=f"t2{dd}_{s}")
                nc.vector.tensor_mul(out=t2[:], in0=sig[:, 16:32],
                                     in1=c_d[:])
                nc.vector.tensor_add(out=c_d[:], in0=t1[:], in1=t2[:])
            tct = wk.tile([128, 16], f32, tag=f"tct{dd}",
                          name=f"tct{dd}_{s}")
            nc.scalar.activation(out=tct[:], in_=c_d[:], func=AF.Tanh)
            nc.vector.tensor_mul(out=hs[dd][:, 16 * t:16 * t + 16],
                                 in0=sig[:, 32:48], in1=tct[:])


def _emit(nc, tc, d, out_ids, T_, TOK, NBLK):
    # bisection aid: stop after a given stage (embed, l0, l1, proj, vit, full)
    stage = os.environ.get("KERNEL_STAGE", "full")
    order = ["embed", "l0", "l1", "proj", "vit", "full"]
    lvl = order.index(stage)
    with tc.tile_pool(name="gc", bufs=1) as gc:
        rep16_sb = gc.tile_from(d["rep16"][:])
        band_sb = [gc.tile_from(d[f"band{q}"][:], name=f"band{q}sb")
                   for q in range(8)]
        transP_sb = gc.tile_from(d["transP"][:])
        M8_sb = gc.tile_from(d["M8"][:])
        iotaI_sb = gc.tile_from(d["iotaI"][:])
        startRow_sb = gc.tile_from(d["startRow"][:])
        endRow_sb = gc.tile_from(d["endRow"][:])
        ones1_sb = gc.tile_from(d["ones1"][:])
        iotaF_sb = gc.tile_from(d["iotaF"][:])
        iotaJP_sb = gc.tile_from(d["iotaJP"][:])
        ident_sb = gc.tile_from(d["ident16"][:])
        ids_sb = gc.tile_from(d["ids_p"][:])
        woutA_sb = gc.tile_from(d["woutA"][:])
        woutB_sb = gc.tile_from(d["woutB"][:])
        ones16_sb = gc.tile_from(d["ones16"][:])

        outT = gc.tile([BL, T_], f32)
        em2 = gc.tile([128, NBLK * K], f32)
        histAll = gc.tile([128, 4 * T_], f32)
        out_sb = gc.tile([BL, T_], i32)

        transP3 = transP_sb.rearrange("p (g i) -> p g i", g=4)
        iotaI3 = iotaI_sb.rearrange("p (g i) -> p g i", g=4)
        M83 = M8_sb.rearrange("p (g jl) -> p g jl", g=4)

        with tc.tile_pool(name="h1p", bufs=1) as h1p:
            h1F = h1p.tile([128, TOK], f32)
            h1B = h1p.tile([128, TOK], f32)

            # ---------------- embedding gather + layer 0 ----------------
            with tc.tile_pool(name="px", bufs=1) as px, \
                 tc.tile_pool(name="wk0", bufs=3) as wk0:
                xT = px.tile([128, TOK], f32)
                w0 = {dd: (px.tile_from(d[f"wx0{dd}"][:], name=f"wx0{dd}sb"),
                           px.tile_from(d[f"wh0{dd}"][:], name=f"wh0{dd}sb"))
                      for dd in (0, 1)}
                biasK0_sb = px.tile_from(d["biasK0"][:])

                with tc.tile_pool(name="ge", bufs=4) as ge, \
                     tc.tile_pool(name="pe", bufs=2, space="PSUM") as pe:
                    # gather order: both ends first so fwd/bwd start early
                    order = []
                    for k in range((NBLK + 1) // 2):
                        order.append(k)
                        if NBLK - 1 - k != k:
                            order.append(NBLK - 1 - k)
                    for n, k in enumerate(order):
                        g_t = ge.tile([128, 128], f32, tag="g")
                        nc.gpsimd.indirect_dma_start(
                            out=g_t[:], out_offset=None, in_=d["embt"][:],
                            in_offset=IndirectOffsetOnAxis(
                                ap=ids_sb[:, k:k + 1], axis=0))
                        tp = pe.tile([128, 128], f32, tag="tp")
                        nc.tensor.transpose(tp[:], g_t[:], ident_sb[:])
                        dst = xT[:, 128 * k:128 * (k + 1)]
                        if n % 2 == 0:
                            nc.vector.tensor_copy(out=dst, in_=tp[:])
                        else:
                            nc.scalar.copy(out=dst, in_=tp[:])

                if lvl >= 1:
                    with tc.tile_pool(name="zp0", bufs=1,
                                      space="PSUM") as zp0:
                        zz0 = zp0.tile([128, 4096], f32)
                        xp = {dd: [(w0[dd][0], xT)] for dd in (0, 1)}
                        _lstm_layer(nc, wk0, zz0, T_, xp, w0[0][1], w0[1][1],
                                    biasK0_sb, ones16_sb, h1F, h1B)

            # ---------------- layer 1 ----------------
            with tc.tile_pool(name="pw1", bufs=1) as pw1, \
                 tc.tile_pool(name="ph2", bufs=1) as ph2:
                w1 = {dd: (pw1.tile_from(d[f"wxA1{dd}"][:], name=f"wxA1{dd}sb"),
                           pw1.tile_from(d[f"wxB1{dd}"][:], name=f"wxB1{dd}sb"),
                           pw1.tile_from(d[f"wh1{dd}"][:], name=f"wh1{dd}sb"))
                      for dd in (0, 1)}
                biasK1_sb = pw1.tile_from(d["biasK1"][:])
                h2F = ph2.tile([128, TOK], f32)
                h2B = ph2.tile([128, TOK], f32)

                if lvl >= 2:
                    with tc.tile_pool(name="zp1", bufs=1,
                                      space="PSUM") as zp1, \
                         tc.tile_pool(name="wk1", bufs=3) as wk1:
                        zz1 = zp1.tile([128, 4096], f32)
                        xp = {dd: [(w1[dd][0], h1F), (w1[dd][1], h1B)]
                              for dd in (0, 1)}
                        _lstm_layer(nc, wk1, zz1, T_, xp, w1[0][2], w1[1][2],
                                    biasK1_sb, ones16_sb, h2F, h2B)

                # ---------------- emission projection ----------------
                # em2[(t%8)*16+b, (t//8)*32+j]  (j = raw tag index)
                with tc.tile_pool(name="pj", bufs=2, space="PSUM") as pj:
                    for k in range(NBLK if lvl >= 3 else 0):
                        ep = pj.tile([128, K], f32, tag="ep")
                        nc.tensor.matmul(out=ep[:],
                                         lhsT=h2F[:, 128 * k:128 * (k + 1)],
                                         rhs=woutA_sb[:], start=True,
                                         stop=False, skip_group_check=True)
                        nc.tensor.matmul(out=ep[:],
                                         lhsT=h2B[:, 128 * k:128 * (k + 1)],
                                         rhs=woutB_sb[:], start=False,
                                         stop=True, skip_group_check=True)
                        dst = em2[:, K * k:K * (k + 1)]
                        if k % 2 == 0:
                            nc.vector.tensor_copy(out=dst, in_=ep[:])
                        else:
                            nc.scalar.copy(out=dst, in_=ep[:])

        # ---------------- Viterbi forward ----------------
        # State ms [128=(jl,b), 4=g] holds score[b, g*8+jl].  Each step:
        #   Rt = broadcast_g(ms) * M8            (DVE, [128,32])
        #   sr = band[tl] @ em_blk + rep16 @ Rt  (PE -> [128,32] replicated)
        #   cand = sr + transP; ms = max_i cand; hist = argmax_i cand
        hA3 = histAll.rearrange("p (g t) -> p g t", t=T_)
        with tc.tile_pool(name="srp", bufs=2, space="PSUM") as srp, \
             tc.tile_pool(name="vt", bufs=3) as vt:
            nc.vector.memset(hA3[:, :, 0], 0.0)
            prev_ms = None
            for s in range(1, (T_ + 1) if lvl >= 4 else 0):
                sr = srp.tile([128, K], f32, tag="sr")
                tl = (s - 1) % 8
                blk = (s - 1) // 8
                nc.tensor.matmul(out=sr[:],
                                 lhsT=band_sb[tl][:],
                                 rhs=em2[:, K * blk:K * (blk + 1)],
                                 start=True, stop=False,
                                 skip_group_check=True)
                if s == 1:
                    nc.tensor.matmul(out=sr[:], lhsT=ones1_sb[:],
                                     rhs=startRow_sb[:], start=False,
                                     stop=True, skip_group_check=True)
                else:
                    Rt = vt.tile([128, K], f32, tag="Rt", name=f"Rt_{s}")
                    Rt3 = Rt.rearrange("p (g jl) -> p g jl", g=4)
                    msb = prev_ms.rearrange("p (g o) -> p g o", o=1) \
                                 .to_broadcast([128, 4, 8])
                    nc.vector.tensor_mul(out=Rt3, in0=msb, in1=M83)
                    nc.tensor.matmul(out=sr[:], lhsT=rep16_sb[:],
                                     rhs=Rt[:], start=False,
                                     stop=(s != T_), skip_group_check=True)
                if s == T_:
                    # fold end transition scores into the final step
                    nc.tensor.matmul(out=sr[:], lhsT=ones1_sb[:],
                                     rhs=endRow_sb[:], start=False,
                                     stop=True, skip_group_check=True)
                    sf = vt.tile([BL, K], f32, tag="sf")
                    nc.vector.tensor_copy(out=sf[:], in_=sr[0:16, :])
                    mfin = vt.tile([BL, 1], f32, tag="mfin")
                    nc.vector.reduce_max(out=mfin[:], in_=sf[:], axis=AX.X)
                    eqf = vt.tile([BL, K], f32, tag="eqf")
                    nc.vector.tensor_tensor(
                        out=eqf[:], in0=sf[:],
                        in1=mfin[:].to_broadcast([BL, K]), op=ALU.is_equal)
                    eif = vt.tile([BL, K], f32, tag="eif")
                    nc.vector.tensor_mul(out=eif[:], in0=eqf[:],
                                         in1=iotaF_sb[:])
                    nc.vector.reduce_max(out=outT[:, T_ - 1:T_], in_=eif[:],
                                         axis=AX.X)
                    break

                cand = vt.tile([128, 128], f32, tag="cand")
                cand3 = cand.rearrange("p (g i) -> p g i", g=4)
                srb = sr[:].rearrange("p (o i) -> p o i", o=1) \
                           .to_broadcast([128, 4, K])
                nc.vector.tensor_add(out=cand3, in0=srb, in1=transP3)
                ms = vt.tile([128, 4], f32, tag="ms", name=f"ms_{s}")
                nc.vector.reduce_max(out=ms[:], in_=cand3, axis=AX.X)
                eqv = vt.tile([128, 128], f32, tag="eqv")
                eq3 = eqv.rearrange("p (g i) -> p g i", g=4)
                msb2 = ms[:].rearrange("p (g o) -> p g o", o=1) \
                            .to_broadcast([128, 4, K])
                nc.vector.tensor_tensor(out=eq3, in0=cand3, in1=msb2,
                                        op=ALU.is_equal)
                eiv = vt.tile([128, 128], f32, tag="eiv")
                ei3 = eiv.rearrange("p (g i) -> p g i", g=4)
                nc.vector.tensor_mul(out=ei3, in0=eq3, in1=iotaI3)
                nc.vector.reduce_max(out=hA3[:, :, s], in_=ei3, axis=AX.X)
                prev_ms = ms

        # ---------------- backtrace ----------------
        with tc.tile_pool(name="pbt", bufs=1) as pbt, \
             tc.tile_pool(name="bt", bufs=2) as bt:
            if lvl >= 5:
                histAllB = pbt.tile([128, 4 * T_], bf16)
                nc.vector.tensor_copy(out=histAllB[:], in_=histAll[:])
                histB = pbt.tile([BL, 32 * T_], bf16)
                hB4 = histB.rearrange("p (jl g t) -> p jl g t", jl=8, g=4)
                for jl in range(8):
                    src = histAllB[16 * jl:16 * (jl + 1), :] \
                        .rearrange("p (g t) -> p g t", t=T_)
                    nc.sync.dma_start(out=hB4[:, jl], in_=src)

                for s in range(T_ - 2, -1, -1):
                    oh = bt.tile([BL, K], bf16, tag="oh")
                    nc.vector.tensor_scalar(out=oh[:], in0=iotaJP_sb[:],
                                            scalar1=outT[:, s + 1:s + 2],
                                            scalar2=None, op0=ALU.is_equal)
                    oh3 = oh.rearrange("p (jl g) -> p jl g", jl=8)
                    scr = bt.tile([BL, K], bf16, tag="scr")
                    scr3 = scr.rearrange("p (jl g) -> p jl g", jl=8)
                    nc.vector.tensor_mul(out=scr3, in0=oh3,
                                         in1=hB4[:, :, :, s + 1])
                    nc.vector.reduce_max(out=outT[:, s:s + 1], in_=scr[:],
                                         axis=AX.X)

                nc.vector.tensor_copy(out=out_sb[:], in_=outT[:])
            else:
                nc.vector.memset(out_sb[:], 0)
            nc.sync.dma_start(out=out_ids[:], in_=out_sb[:])


def _run(inputs_np, consts, T_):
    global LAST_RESULTS
    nc = _build_program(T_)
    in_maps = []
    for core in range(NCORES):
        m = dict(consts)
        m["ids_p"] = _ids_for_core(inputs_np, core, T_)
        in_maps.append(m)
    trace = bool(int(os.environ.get("KERNEL_TRACE", "0")))
    res = bass_utils.run_bass_kernel_spmd(
        nc, in_maps, core_ids=list(range(NCORES)), trace=trace)
    LAST_RESULTS = res
    return np.concatenate([r["out_ids"] for r in res.results], axis=0)


def kernel(inputs, tags, emb, w_ih_l0, w_hh_l0, b_l0,
           w_ih_l1, w_hh_l1, b_l1, W_out, b_out,
           start_t, end_t, trans, _T=TFULL):
    del tags  # unused at decode time
    inputs_np = np.ascontiguousarray(np.asarray(inputs, dtype=np.int32))
    consts = _host_consts(emb, w_ih_l0, w_hh_l0, b_l0, w_ih_l1, w_hh_l1,
                          b_l1, W_out, b_out, start_t, end_t, trans)
    return _run(inputs_np, consts, _T)


# revision 11
# speedup vs baseline: 3.1094x; 1.6845x over previous
"""BiLSTM-CRF decode kernel for Trainium2 (8 NeuronCores, batch-sharded).

Model: embedding lookup -> 2-layer BiLSTM (H=128/dir) -> linear -> CRF Viterbi.
Output: [B, T] int32 best-path tags.

Sharding: data-parallel over batch, B=128 -> 16 rows per core. Feature-major
layout ([feature partitions, batch free]) so the serial time recurrences run
full-width engine ops.

Perf structure (v2):
- The input-to-gates GEMMs (x @ Wih) are hoisted out of the recurrence into
  bulk N=256 matmuls that pre-accumulate zx+bias into PSUM half-banks; the
  in-loop recurrence only adds Whh @ h into the same PSUM cells. The gate
  nonlinearities read PSUM directly.
- Precision split (fp16 matmuls are 1-pass on the PE, fp32 are 2-pass):
  layer-0 recurrence and layer-1 input GEMMs run fp16 (h1 stored fp16);
  layer-1 recurrence and the emission projection stay fp32 (h2 stored fp32),
  which reproduces the reference tags exactly.  Making l1/proj fp16 was
  measured at 60/65536 tag flips (rel err 2.1e-2) -- just over the gate.
- Viterbi forward uses a replicated-score formulation: one masked DVE mul +
  one rep16 matmul replaces the 8 per-step band broadcasts.
"""

import os
import numpy as np

import concourse.bass as bass
import concourse.bacc as bacc
import concourse.tile as tile
import concourse.mybir as mybir
from concourse.bass import IndirectOffsetOnAxis
from concourse import bass_utils

B, TFULL, V, D, H, K = 128, 512, 50000, 128, 128, 32
NCORES = 8
BL = B // NCORES  # 16 batch rows per core

f32 = mybir.dt.float32
f16 = mybir.dt.float16
bf16 = mybir.dt.bfloat16
i32 = mybir.dt.int32
AF = mybir.ActivationFunctionType
ALU = mybir.AluOpType
AX = mybir.AxisListType

# torch gate order is [i, f, g, o]; we use [i, f, o, g] so the sigmoid gates
# (i, f, o) are contiguous and tanh(g) is the last chunk.
_PERM = np.r_[0:H, H:2 * H, 3 * H:4 * H, 2 * H:3 * H]

LAST_RESULTS = None  # BassKernelResults of the most recent run (for test.py)


def _f(x):
    return np.ascontiguousarray(np.asarray(x, dtype=np.float32))


def _h(x):
    return np.ascontiguousarray(np.asarray(x, dtype=np.float16))


def _host_consts(emb, w_ih_l0, w_hh_l0, b_l0, w_ih_l1, w_hh_l1, b_l1,
                 W_out, b_out, start_t, end_t, trans):
    """Build all per-core-identical device input arrays."""
    c = {}
    c["embt"] = _h(emb)

    for d in (0, 1):
        c[f"wx0{d}"] = _h(np.asarray(w_ih_l0)[d][_PERM].T)      # [128, 512]
        c[f"wh0{d}"] = _h(np.asarray(w_hh_l0)[d][_PERM].T)      # [128, 512]
        w1 = np.asarray(w_ih_l1)[d][_PERM]                       # [512, 256]
        c[f"wxA1{d}"] = _h(w1[:, :H].T)                          # [128, 512]
        c[f"wxB1{d}"] = _h(w1[:, H:].T)                          # [128, 512]
        c[f"wh1{d}"] = _h(np.asarray(w_hh_l1)[d][_PERM].T)       # [128, 512]

    for l, bl in ((0, b_l0), (1, b_l1)):
        bk = np.zeros((1, 1024), dtype=np.float16)
        for d in (0, 1):
            bperm = np.asarray(bl)[d][_PERM]
            bk[0, d * 512:(d + 1) * 512] = bperm
        c[f"biasK{l}"] = np.ascontiguousarray(bk)
    c["ones16"] = np.ones((1, 512), dtype=np.float16)

    # Emission projection (b_out folded into trans/start below).
    WoT = _f(np.asarray(W_out).T)                                # [256, 32]
    c["woutA"] = _f(WoT[:H])
    c["woutB"] = _f(WoT[H:])

    km = np.arange(128)
    rep_full = (km[:, None] % 16 == km[None, :] % 16).astype(np.float32)
    c["rep16"] = _f(rep_full)
    for q in range(8):
        bm = rep_full.copy()
        bm[(km // 16) != q, :] = 0.0
        c[f"band{q}"] = _f(bm)

    # Viterbi, tag layout: score column i = tag i; per-partition-block jl
    # owns tags j = g*8 + jl (g in 0..3) stored as ms[:, g].
    trans_p = _f(trans)                                          # [32, 32]
    jlv = np.arange(128) // 16                                   # [128]
    gv = np.arange(4)
    # transP[(jl,b), g*32 + i] = trans_p[i, g*8 + jl]
    tp = np.zeros((128, 128), dtype=np.float32)
    for p in range(128):
        for g in range(4):
            tp[p, g * 32:(g + 1) * 32] = trans_p[:, g * 8 + jlv[p]]
    c["transP"] = tp
    # M8[(jl2,b), (g,jl)] = (jl2 == jl)
    m8 = np.zeros((128, 32), dtype=np.float32)
    for p in range(128):
        for g in range(4):
            for jl in range(8):
                m8[p, g * 8 + jl] = 1.0 if (p // 16) == jl else 0.0
    c["M8"] = m8
    c["iotaI"] = _f(np.tile(np.arange(32, dtype=np.float32), (128, 4)))
    c["startRow"] = _f(np.asarray(start_t))[None, :]
    c["bo1"] = _f(np.asarray(b_out))[None, :]
    c["endRow"] = _f(np.asarray(end_t))[None, :]
    c["ones1"] = np.ones((1, 128), dtype=np.float32)
    c["iotaF"] = _f(np.tile(np.arange(32, dtype=np.float32), (BL, 1)))

    jp = np.empty(32, dtype=np.float32)
    for jl in range(8):
        for g in range(4):
            jp[jl * 4 + g] = g * 8 + jl
    c["iotaJP"] = _f(np.tile(jp, (BL, 1)))                       # [16, 32]

    c["ident16"] = np.eye(128, dtype=np.float16)
    return c


def _ids_for_core(inputs_np, core, T_):
    ids_c = inputs_np[core * BL:(core + 1) * BL, :T_]            # [16, T]
    flat = np.ascontiguousarray(ids_c.T).reshape(-1)             # t-major
    nblk = (BL * T_) // 128
    return np.ascontiguousarray(flat.reshape(nblk, 128).T.astype(np.int32))


def _build_program(T_):
    """Build the full single-core Bass program (identical across cores)."""
    TOK = BL * T_
    NBLK = TOK // 128

    nc = bacc.Bacc()
    d = {}

    def din(name, shape, dtype=f32):
        d[name] = nc.dram_tensor(name, list(shape), dtype, kind="ExternalInput")
        return d[name]

    din("ids_p", [128, NBLK], i32)
    din("embt", [V, D], f16)
    for dd in (0, 1):
        din(f"wx0{dd}", [128, 512], f16)
        din(f"wh0{dd}", [128, 512], f16)
        din(f"wxA1{dd}", [128, 512], f16)
        din(f"wxB1{dd}", [128, 512], f16)
        din(f"wh1{dd}", [128, 512], f16)
    din("biasK0", [1, 1024], f16)
    din("biasK1", [1, 1024], f16)
    din("ones16", [1, 512], f16)
    din("woutA", [128, K])
    din("woutB", [128, K])
    din("rep16", [128, 128])
    for q in range(8):
        din(f"band{q}", [128, 128])
    din("transP", [128, 128])
    din("M8", [128, K])
    din("iotaI", [128, 128])
    din("startRow", [1, K])
    din("bo1", [1, K])
    din("endRow", [1, K])
    din("ones1", [1, 128])
    din("iotaF", [BL, K])
    din("iotaJP", [BL, K])
    din("ident16", [128, 128], f16)
    out_ids = nc.dram_tensor("out_ids", [BL, T_], i32, kind="ExternalOutput")

    with tile.TileContext(nc) as tc:
        _emit(nc, tc, d, out_ids, T_, TOK, NBLK)
    nc.compile()
    return nc


def _lstm_layer(nc, wk, zz, T_, xparts, wh_f, wh_b, biasK_sb, ones16_sb,
                hF, hB, hs32=None):
    """One BiLSTM layer; fwd and bwd directions interleaved per step.

    zz: PSUM tile [128, 4096] = 8 banks; bank (dd*4+ci) holds
    [buf(2), tb(16), b(16)] columns. Bulk zx+bias matmuls pre-fill 16-step
    half-banks; the in-loop Whh matmuls accumulate into single-step cells.
    xparts: per-dir list of (wxT [128,512] f16 sbuf, rhs [128, TOK] f16 sbuf).
    hF/hB: [128, TOK] f16 output buffers (also the recurrent matmul input).
    """
    NBLKS = T_ // 16
    zz6 = zz.rearrange("p (dd ci buf tb b) -> p dd ci buf tb b",
                       dd=2, ci=4, buf=2, tb=16)

    def off(dd, ci, buf):
        return ((dd * 4 + ci) * 2 + buf) * 256

    def bulk(dd, blk):
        buf = blk % 2
        for ci in range(4):
            o = zz[:, off(dd, ci, buf):off(dd, ci, buf) + 256]
            bsl = biasK_sb[:, (dd * 4 + ci) * 128:(dd * 4 + ci + 1) * 128]
            nc.tensor.matmul(out=o, lhsT=bsl,
                             rhs=ones16_sb[:, 0:256], start=True, stop=False,
                             skip_group_check=True)
            for (wT, rhs) in xparts[dd]:
                nc.tensor.matmul(out=o, lhsT=wT[:, ci * 128:(ci + 1) * 128],
                                 rhs=rhs[:, 256 * blk:256 * (blk + 1)],
                                 start=False, stop=False,
                                 skip_group_check=True)

    bulk(0, 0)
    bulk(1, NBLKS - 1)

    cF = wk.tile([128, 16], f32, tag="cF")
    cB = wk.tile([128, 16], f32, tag="cB")
    cs = (cF, cB)
    whs = (wh_f, wh_b)
    hs = (hF, hB)

    for s in range(T_):
        if s % 16 == 0 and s > 0:
            k = s // 16
            if k < NBLKS:
                bulk(0, k)
                bulk(1, NBLKS - 1 - k)
        tf, tb_t = s, T_ - 1 - s
        for dd, t in ((0, tf), (1, tb_t)):
            if s > 0:
                tp = t - 1 if dd == 0 else t + 1
                buf, tbi = (t // 16) % 2, t % 16
                wh = whs[dd]
                for ci in range(4):
                    o = zz[:, off(dd, ci, buf) + tbi * 16:
                           off(dd, ci, buf) + tbi * 16 + 16]
                    nc.tensor.matmul(out=o,
                                     lhsT=wh[:, ci * 128:(ci + 1) * 128],
                                     rhs=hs[dd][:, 16 * tp:16 * tp + 16],
                                     start=False, stop=True,
                                     skip_group_check=True)

        # fwd and bwd get fully separate ACT/DVE chains so the scheduler can
        # pipeline one direction's gates against the other's matmuls.
        for dd, t in ((0, tf), (1, tb_t)):
            buf, tbi = (t // 16) % 2, t % 16
            c_d = cs[dd]
            sig = wk.tile([128, 48], f32, tag=f"sig{dd}",
                          name=f"sig{dd}_{s}")
            nc.scalar.activation(out=sig[:], in_=zz6[:, dd, 0:3, buf, tbi],
                                 func=AF.Sigmoid)
            tg = wk.tile([128, 16], f32, tag=f"tg{dd}", name=f"tg{dd}_{s}")
            nc.scalar.activation(out=tg[:], in_=zz6[:, dd, 3, buf, tbi],
                                 func=AF.Tanh)
            if s == 0:
                nc.vector.tensor_mul(out=c_d[:], in0=sig[:, 0:16], in1=tg[:])
            else:
                t1 = wk.tile([128, 16], f32, tag=f"t1{dd}",
                             name=f"t1{dd}_{s}")
                nc.vector.tensor_mul(out=t1[:], in0=sig[:, 0:16], in1=tg[:])
                t2 = wk.tile([128, 16], f32, tag=f"t2{dd}",
                             name=f"t2{dd}_{s}")
                nc.vector.tensor_mul(out=t2[:], in0=sig[:, 16:32],
                                     in1=c_d[:])
                nc.vector.tensor_add(out=c_d[:], in0=t1[:], in1=t2[:])
            tct = wk.tile([128, 16], f32, tag=f"tct{dd}",
                          name=f"tct{dd}_{s}")
            nc.scalar.activation(out=tct[:], in_=c_d[:], func=AF.Tanh)
            nc.vector.tensor_mul(out=hs[dd][:, 16 * t:16 * t + 16],
                                 in0=sig[:, 32:48], in1=tct[:])
            if hs32 is not None and hs32[dd] is not None:
                nc.vector.tensor_mul(out=hs32[dd][:, 16 * t:16 * t + 16],
                                     in0=sig[:, 32:48], in1=tct[:])


def _emit(nc, tc, d, out_ids, T_, TOK, NBLK):
    # bisection aid: stop after a given stage (embed, l0, l1, proj, vit, full)
    stage = os.environ.get("KERNEL_STAGE", "full")
    order = ["embed", "l0", "l1", "proj", "vit", "full"]
    lvl = order.index(stage)
    with tc.tile_pool(name="gc", bufs=1) as gc:
        rep16_sb = gc.tile_from(d["rep16"][:])
        band_sb = [gc.tile_from(d[f"band{q}"][:], name=f"band{q}sb")
                   for q in range(8)]
        transP_sb = gc.tile_from(d["transP"][:])
        M8_sb = gc.tile_from(d["M8"][:])
        iotaI_sb = gc.tile_from(d["iotaI"][:])
        startRow_sb = gc.tile_from(d["startRow"][:])
        endRow_sb = gc.tile_from(d["endRow"][:])
        ones1_sb = gc.tile_from(d["ones1"][:])
        iotaF_sb = gc.tile_from(d["iotaF"][:])
        iotaJP_sb = gc.tile_from(d["iotaJP"][:])
        ident_sb = gc.tile_from(d["ident16"][:])
        ids_sb = gc.tile_from(d["ids_p"][:])
        woutA_sb = gc.tile_from(d["woutA"][:])
        woutB_sb = gc.tile_from(d["woutB"][:])
        ones16_sb = gc.tile_from(d["ones16"][:])
        bo1_sb = gc.tile_from(d["bo1"][:])

        outT = gc.tile([BL, T_], f32)
        em2 = gc.tile([128, NBLK * K], f32)
        histAll = gc.tile([128, 4 * T_], f32)
        out_sb = gc.tile([BL, T_], i32)

        transP3 = transP_sb.rearrange("p (g i) -> p g i", g=4)
        iotaI3 = iotaI_sb.rearrange("p (g i) -> p g i", g=4)
        M83 = M8_sb.rearrange("p (g jl) -> p g jl", g=4)

        with tc.tile_pool(name="h1p", bufs=1) as h1p:
            h1F = h1p.tile([128, TOK], f16)
            h1B = h1p.tile([128, TOK], f16)

            # ---------------- embedding gather + layer 0 ----------------
            with tc.tile_pool(name="px", bufs=1) as px, \
                 tc.tile_pool(name="wk0", bufs=3) as wk0:
                xT = px.tile([128, TOK], f16)
                w0 = {dd: (px.tile_from(d[f"wx0{dd}"][:], name=f"wx0{dd}sb"),
                           px.tile_from(d[f"wh0{dd}"][:], name=f"wh0{dd}sb"))
                      for dd in (0, 1)}
                biasK0_sb = px.tile_from(d["biasK0"][:])

                with tc.tile_pool(name="ge", bufs=4) as ge, \
                     tc.tile_pool(name="pe", bufs=2, space="PSUM") as pe:
                    # gather order: both ends first so fwd/bwd start early
                    order = []
                    for k in range((NBLK + 1) // 2):
                        order.append(k)
                        if NBLK - 1 - k != k:
                            order.append(NBLK - 1 - k)
                    for n, k in enumerate(order):
                        g_t = ge.tile([128, 128], f16, tag="g")
                        nc.gpsimd.indirect_dma_start(
                            out=g_t[:], out_offset=None, in_=d["embt"][:],
                            in_offset=IndirectOffsetOnAxis(
                                ap=ids_sb[:, k:k + 1], axis=0))
                        tp = pe.tile([128, 128], f16, tag="tp")
                        nc.tensor.transpose(tp[:], g_t[:], ident_sb[:])
                        dst = xT[:, 128 * k:128 * (k + 1)]
                        if n % 2 == 0:
                            nc.vector.tensor_copy(out=dst, in_=tp[:])
                        else:
                            nc.scalar.copy(out=dst, in_=tp[:])

                if lvl >= 1:
                    with tc.tile_pool(name="zp0", bufs=1,
                                      space="PSUM") as zp0:
                        zz0 = zp0.tile([128, 4096], f32)
                        xp = {dd: [(w0[dd][0], xT)] for dd in (0, 1)}
                        _lstm_layer(nc, wk0, zz0, T_, xp, w0[0][1], w0[1][1],
                                    biasK0_sb, ones16_sb, h1F, h1B)

            # ---------------- layer 1 ----------------
            with tc.tile_pool(name="pw1", bufs=1) as pw1, \
                 tc.tile_pool(name="ph2", bufs=1) as ph2:
                w1 = {dd: (pw1.tile_from(d[f"wxA1{dd}"][:], name=f"wxA1{dd}sb"),
                           pw1.tile_from(d[f"wxB1{dd}"][:], name=f"wxB1{dd}sb"),
                           pw1.tile_from(d[f"wh1{dd}"][:], name=f"wh1{dd}sb"))
                      for dd in (0, 1)}
                biasK1_sb = pw1.tile_from(d["biasK1"][:])
                h2F = ph2.tile([128, TOK], f16)
                h2F32 = ph2.tile([128, TOK], f32)
                h2B = ph2.tile([128, TOK], f16)
                h2B32 = ph2.tile([128, TOK], f32)

                if lvl >= 2:
                    with tc.tile_pool(name="zp1", bufs=1,
                                      space="PSUM") as zp1, \
                         tc.tile_pool(name="wk1", bufs=3) as wk1:
                        zz1 = zp1.tile([128, 4096], f32)
                        xp = {dd: [(w1[dd][0], h1F), (w1[dd][1], h1B)]
                              for dd in (0, 1)}
                        _lstm_layer(nc, wk1, zz1, T_, xp, w1[0][2], w1[1][2],
                                    biasK1_sb, ones16_sb, h2F, h2B,
                                    hs32=(h2F32, h2B32))

                # ---------------- emission projection ----------------
                # em2[(t%8)*16+b, (t//8)*32+j]  (j = raw tag index)
                with tc.tile_pool(name="pj", bufs=2, space="PSUM") as pj:
                    for k in range(NBLK if lvl >= 3 else 0):
                        ep = pj.tile([128, K], f32, tag="ep")
                        nc.tensor.matmul(out=ep[:], lhsT=ones1_sb[:],
                                         rhs=bo1_sb[:], start=True,
                                         stop=False, skip_group_check=True)
                        nc.tensor.matmul(out=ep[:],
                                         lhsT=h2F32[:, 128 * k:128 * (k + 1)],
                                         rhs=woutA_sb[:], start=False,
                                         stop=False, skip_group_check=True)
                        nc.tensor.matmul(out=ep[:],
                                         lhsT=h2B32[:, 128 * k:128 * (k + 1)],
                                         rhs=woutB_sb[:], start=False,
                                         stop=True, skip_group_check=True)
                        dst = em2[:, K * k:K * (k + 1)]
                        if k % 2 == 0:
                            nc.vector.tensor_copy(out=dst, in_=ep[:])
                        else:
                            nc.scalar.copy(out=dst, in_=ep[:])

        # ---------------- Viterbi forward ----------------
        # State ms [128=(jl,b), 4=g] holds score[b, g*8+jl].  Each step:
        #   Rt = broadcast_g(ms) * M8            (DVE, [128,32])
        #   sr = band[tl] @ em_blk + rep16 @ Rt  (PE -> [128,32] replicated)
        #   cand = sr + transP; ms = max_i cand; hist = argmax_i cand
        hA3 = histAll.rearrange("p (g t) -> p g t", t=T_)
        with tc.tile_pool(name="srp", bufs=2, space="PSUM") as srp, \
             tc.tile_pool(name="vt", bufs=3) as vt:
            nc.vector.memset(hA3[:, :, 0], 0.0)
            prev_ms = None
            for s in range(1, (T_ + 1) if lvl >= 4 else 0):
                sr = srp.tile([128, K], f32, tag="sr")
                tl = (s - 1) % 8
                blk = (s - 1) // 8
                nc.tensor.matmul(out=sr[:],
                                 lhsT=band_sb[tl][:],
                                 rhs=em2[:, K * blk:K * (blk + 1)],
                                 start=True, stop=False,
                                 skip_group_check=True)
                if s == 1:
                    nc.tensor.matmul(out=sr[:], lhsT=ones1_sb[:],
                                     rhs=startRow_sb[:], start=False,
                                     stop=True, skip_group_check=True)
                else:
                    Rt = vt.tile([128, K], f32, tag="Rt", name=f"Rt_{s}")
                    Rt3 = Rt.rearrange("p (g jl) -> p g jl", g=4)
                    msb = prev_ms.rearrange("p (g o) -> p g o", o=1) \
                                 .to_broadcast([128, 4, 8])
                    nc.vector.tensor_mul(out=Rt3, in0=msb, in1=M83)
                    nc.tensor.matmul(out=sr[:], lhsT=rep16_sb[:],
                                     rhs=Rt[:], start=False,
                                     stop=(s != T_), skip_group_check=True)
                if s == T_:
                    # fold end transition scores into the final step
                    nc.tensor.matmul(out=sr[:], lhsT=ones1_sb[:],
                                     rhs=endRow_sb[:], start=False,
                                     stop=True, skip_group_check=True)
                    sf = vt.tile([BL, K], f32, tag="sf")
                    nc.vector.tensor_copy(out=sf[:], in_=sr[0:16, :])
                    mfin = vt.tile([BL, 1], f32, tag="mfin")
                    nc.vector.reduce_max(out=mfin[:], in_=sf[:], axis=AX.X)
                    eqf = vt.tile([BL, K], f32, tag="eqf")
                    nc.vector.tensor_tensor(
                        out=eqf[:], in0=sf[:],
                        in1=mfin[:].to_broadcast([BL, K]), op=ALU.is_equal)
                    eif = vt.tile([BL, K], f32, tag="eif")
                    nc.vector.tensor_mul(out=eif[:], in0=eqf[:],
                                         in1=iotaF_sb[:])
                    nc.vector.reduce_max(out=outT[:, T_ - 1:T_], in_=eif[:],
                                         axis=AX.X)
                    break

                cand = vt.tile([128, 128], f32, tag="cand")
                cand3 = cand.rearrange("p (g i) -> p g i", g=4)
                srb = sr[:].rearrange("p (o i) -> p o i", o=1) \
                           .to_broadcast([128, 4, K])
                nc.vector.tensor_add(out=cand3, in0=srb, in1=transP3)
                ms = vt.tile([128, 4], f32, tag="ms", name=f"ms_{s}")
                nc.vector.reduce_max(out=ms[:], in_=cand3, axis=AX.X)
                eqv = vt.tile([128, 128], f32, tag="eqv")
                eq3 = eqv.rearrange("p (g i) -> p g i", g=4)
                msb2 = ms[:].rearrange("p (g o) -> p g o", o=1) \
                            .to_broadcast([128, 4, K])
                nc.vector.tensor_tensor(out=eq3, in0=cand3, in1=msb2,
                                        op=ALU.is_equal)
                eiv = vt.tile([128, 128], f32, tag="eiv")
                ei3 = eiv.rearrange("p (g i) -> p g i", g=4)
                nc.vector.tensor_mul(out=ei3, in0=eq3, in1=iotaI3)
                nc.vector.reduce_max(out=hA3[:, :, s], in_=ei3, axis=AX.X)
                prev_ms = ms

        # ---------------- backtrace ----------------
        with tc.tile_pool(name="pbt", bufs=1) as pbt, \
             tc.tile_pool(name="bt", bufs=2) as bt:
            if lvl >= 5:
                histAllB = pbt.tile([128, 4 * T_], bf16)
                nc.vector.tensor_copy(out=histAllB[:], in_=histAll[:])
                histB = pbt.tile([BL, 32 * T_], bf16)
                hB4 = histB.rearrange("p (jl g t) -> p jl g t", jl=8, g=4)
                for jl in range(8):
                    src = histAllB[16 * jl:16 * (jl + 1), :] \
                        .rearrange("p (g t) -> p g t", t=T_)
                    nc.sync.dma_start(out=hB4[:, jl], in_=src)

                for s in range(T_ - 2, -1, -1):
                    oh = bt.tile([BL, K], bf16, tag="oh")
                    nc.vector.tensor_scalar(out=oh[:], in0=iotaJP_sb[:],
                                            scalar1=outT[:, s + 1:s + 2],
                                            scalar2=None, op0=ALU.is_equal)
                    oh3 = oh.rearrange("p (jl g) -> p jl g", jl=8)
                    scr = bt.tile([BL, K], bf16, tag="scr")
                    scr3 = scr.rearrange("p (jl g) -> p jl g", jl=8)
                    nc.vector.tensor_mul(out=scr3, in0=oh3,
                                         in1=hB4[:, :, :, s + 1])
                    nc.vector.reduce_max(out=outT[:, s:s + 1], in_=scr[:],
                                         axis=AX.X)

                nc.vector.tensor_copy(out=out_sb[:], in_=outT[:])
            else:
                nc.vector.memset(out_sb[:], 0)
            nc.sync.dma_start(out=out_ids[:], in_=out_sb[:])


def _run(inputs_np, consts, T_):
    global LAST_RESULTS
    nc = _build_program(T_)
    in_maps = []
    for core in range(NCORES):
        m = dict(consts)
        m["ids_p"] = _ids_for_core(inputs_np, core, T_)
        in_maps.append(m)
    trace = bool(int(os.environ.get("KERNEL_TRACE", "0")))
    res = bass_utils.run_bass_kernel_spmd(
        nc, in_maps, core_ids=list(range(NCORES)), trace=trace)
    LAST_RESULTS = res
    return np.concatenate([r["out_ids"] for r in res.results], axis=0)


def kernel(inputs, tags, emb, w_ih_l0, w_hh_l0, b_l0,
           w_ih_l1, w_hh_l1, b_l1, W_out, b_out,
           start_t, end_t, trans, _T=TFULL):
    del tags  # unused at decode time
    inputs_np = np.ascontiguousarray(np.asarray(inputs, dtype=np.int32))
    consts = _host_consts(emb, w_ih_l0, w_hh_l0, b_l0, w_ih_l1, w_hh_l1,
                          b_l1, W_out, b_out, start_t, end_t, trans)
    return _run(inputs_np, consts, _T)


# revision 15
# speedup vs baseline: 3.2565x; 1.0473x over previous
"""BiLSTM-CRF decode kernel for Trainium2 (8 NeuronCores, batch-sharded).

Model: embedding lookup -> 2-layer BiLSTM (H=128/dir) -> linear -> CRF Viterbi.
Output: [B, T] int32 best-path tags.

Sharding: data-parallel over batch, B=128 -> 16 rows per core. Feature-major
layout ([feature partitions, batch free]) so the serial time recurrences run
full-width engine ops.

Perf structure (v2):
- The input-to-gates GEMMs (x @ Wih) are hoisted out of the recurrence into
  bulk N=256 matmuls that pre-accumulate zx+bias into PSUM half-banks; the
  in-loop recurrence only adds Whh @ h into the same PSUM cells. The gate
  nonlinearities read PSUM directly.
- Precision split (fp16 matmuls are 1-pass on the PE, fp32 are 2-pass):
  everything runs fp16 except the l1-backward g-gate recurrence matmul and
  the emission projection, which stay fp32 (h2 kept in fp16+fp32 copies).
  The g-gate dominates error injection (dc/dz_g ~ 0.5 vs ~0.06 for i/f/o):
  fp16 i/f/o in l1-bwd adds zero tag flips, fp16 g there adds ~36.
  Measured 24/65536 flips (rel err 1.33e-2) vs the 2e-2 gate.
- Viterbi forward uses a replicated-score formulation: one masked DVE mul +
  one rep16 matmul replaces the 8 per-step band broadcasts.
"""

import os
import numpy as np

import concourse.bass as bass
import concourse.bacc as bacc
import concourse.tile as tile
import concourse.mybir as mybir
from concourse.bass import IndirectOffsetOnAxis
from concourse import bass_utils

B, TFULL, V, D, H, K = 128, 512, 50000, 128, 128, 32
NCORES = 8
BL = B // NCORES  # 16 batch rows per core

f32 = mybir.dt.float32
f16 = mybir.dt.float16
bf16 = mybir.dt.bfloat16
i32 = mybir.dt.int32
AF = mybir.ActivationFunctionType
ALU = mybir.AluOpType
AX = mybir.AxisListType

# torch gate order is [i, f, g, o]; we use [i, f, o, g] so the sigmoid gates
# (i, f, o) are contiguous and tanh(g) is the last chunk.
_PERM = np.r_[0:H, H:2 * H, 3 * H:4 * H, 2 * H:3 * H]

LAST_RESULTS = None  # BassKernelResults of the most recent run (for test.py)


def _f(x):
    return np.ascontiguousarray(np.asarray(x, dtype=np.float32))


def _h(x):
    return np.ascontiguousarray(np.asarray(x, dtype=np.float16))


def _host_consts(emb, w_ih_l0, w_hh_l0, b_l0, w_ih_l1, w_hh_l1, b_l1,
                 W_out, b_out, start_t, end_t, trans):
    """Build all per-core-identical device input arrays."""
    c = {}
    c["embt"] = _h(emb)

    for d in (0, 1):
        c[f"wx0{d}"] = _h(np.asarray(w_ih_l0)[d][_PERM].T)      # [128, 512]
        c[f"wh0{d}"] = _h(np.asarray(w_hh_l0)[d][_PERM].T)      # [128, 512]
        w1 = np.asarray(w_ih_l1)[d][_PERM]                       # [512, 256]
        c[f"wxA1{d}"] = _h(w1[:, :H].T)                          # [128, 512]
        c[f"wxB1{d}"] = _h(w1[:, H:].T)                          # [128, 512]
        c[f"wh1{d}"] = _h(np.asarray(w_hh_l1)[d][_PERM].T)       # [128, 512]

    for l, bl in ((0, b_l0), (1, b_l1)):
        bk = np.zeros((1, 1024), dtype=np.float16)
        for d in (0, 1):
            bperm = np.asarray(bl)[d][_PERM]
            bk[0, d * 512:(d + 1) * 512] = bperm
        c[f"biasK{l}"] = np.ascontiguousarray(bk)
    c["ones16"] = np.ones((1, 512), dtype=np.float16)

    # Emission projection (b_out folded into trans/start below).
    WoT = _f(np.asarray(W_out).T)                                # [256, 32]
    c["woutA"] = _f(WoT[:H])
    c["woutB"] = _f(WoT[H:])

    km = np.arange(128)
    rep_full = (km[:, None] % 16 == km[None, :] % 16).astype(np.float32)
    c["rep16"] = _f(rep_full)
    for q in range(8):
        bm = rep_full.copy()
        bm[(km // 16) != q, :] = 0.0
        c[f"band{q}"] = _f(bm)

    # Viterbi, tag layout: score column i = tag i; per-partition-block jl
    # owns tags j = g*8 + jl (g in 0..3) stored as ms[:, g].
    trans_p = _f(trans)                                          # [32, 32]
    jlv = np.arange(128) // 16                                   # [128]
    gv = np.arange(4)
    # transP[(jl,b), g*32 + i] = trans_p[i, g*8 + jl]
    tp = np.zeros((128, 128), dtype=np.float32)
    for p in range(128):
        for g in range(4):
            tp[p, g * 32:(g + 1) * 32] = trans_p[:, g * 8 + jlv[p]]
    c["transP"] = tp
    # M8[(jl2,b), (g,jl)] = (jl2 == jl)
    m8 = np.zeros((128, 32), dtype=np.float32)
    for p in range(128):
        for g in range(4):
            for jl in range(8):
                m8[p, g * 8 + jl] = 1.0 if (p // 16) == jl else 0.0
    c["M8"] = m8
    c["iotaI"] = _f(np.tile(np.arange(32, dtype=np.float32), (128, 4)))
    c["startRow"] = _f(np.asarray(start_t))[None, :]
    c["bo1"] = _f(np.asarray(b_out))[None, :]
    c["endRow"] = _f(np.asarray(end_t))[None, :]
    c["ones1"] = np.ones((1, 128), dtype=np.float32)
    c["iotaF"] = _f(np.tile(np.arange(32, dtype=np.float32), (BL, 1)))

    jp = np.empty(32, dtype=np.float32)
    for jl in range(8):
        for g in range(4):
            jp[jl * 4 + g] = g * 8 + jl
    c["iotaJP"] = _f(np.tile(jp, (BL, 1)))                       # [16, 32]

    c["ident16"] = np.eye(128, dtype=np.float16)
    return c


def _ids_for_core(inputs_np, core, T_):
    ids_c = inputs_np[core * BL:(core + 1) * BL, :T_]            # [16, T]
    flat = np.ascontiguousarray(ids_c.T).reshape(-1)             # t-major
    nblk = (BL * T_) // 128
    return np.ascontiguousarray(flat.reshape(nblk, 128).T.astype(np.int32))


def _build_program(T_):
    """Build the full single-core Bass program (identical across cores)."""
    TOK = BL * T_
    NBLK = TOK // 128

    nc = bacc.Bacc()
    d = {}

    def din(name, shape, dtype=f32):
        d[name] = nc.dram_tensor(name, list(shape), dtype, kind="ExternalInput")
        return d[name]

    din("ids_p", [128, NBLK], i32)
    din("embt", [V, D], f16)
    for dd in (0, 1):
        din(f"wx0{dd}", [128, 512], f16)
        din(f"wh0{dd}", [128, 512], f16)
        din(f"wxA1{dd}", [128, 512], f16)
        din(f"wxB1{dd}", [128, 512], f16)
        din(f"wh1{dd}", [128, 512], f16)
    din("biasK0", [1, 1024], f16)
    din("biasK1", [1, 1024], f16)
    din("ones16", [1, 512], f16)
    din("woutA", [128, K])
    din("woutB", [128, K])
    din("rep16", [128, 128])
    for q in range(8):
        din(f"band{q}", [128, 128])
    din("transP", [128, 128])
    din("M8", [128, K])
    din("iotaI", [128, 128])
    din("startRow", [1, K])
    din("bo1", [1, K])
    din("endRow", [1, K])
    din("ones1", [1, 128])
    din("iotaF", [BL, K])
    din("iotaJP", [BL, K])
    din("ident16", [128, 128], f16)
    out_ids = nc.dram_tensor("out_ids", [BL, T_], i32, kind="ExternalOutput")

    with tile.TileContext(nc) as tc:
        _emit(nc, tc, d, out_ids, T_, TOK, NBLK)
    nc.compile()
    return nc


def _lstm_layer(nc, wk, zz, T_, xparts, wh_f, wh_b, biasK_sb, ones16_sb,
                hF, hB, hs32=None):
    """One BiLSTM layer; fwd and bwd directions interleaved per step.

    zz: PSUM tile [128, 4096] = 8 banks; bank (dd*4+ci) holds
    [buf(2), tb(16), b(16)] columns. Bulk zx+bias matmuls pre-fill 16-step
    half-banks; the in-loop Whh matmuls accumulate into single-step cells.
    xparts: per-dir list of (wxT [128,512] f16 sbuf, rhs [128, TOK] f16 sbuf).
    hF/hB: [128, TOK] f16 output buffers (also the recurrent matmul input).
    """
    NBLKS = T_ // 16
    zz6 = zz.rearrange("p (dd ci buf tb b) -> p dd ci buf tb b",
                       dd=2, ci=4, buf=2, tb=16)

    def off(dd, ci, buf):
        return ((dd * 4 + ci) * 2 + buf) * 256

    def bulk(dd, blk):
        buf = blk % 2
        for ci in range(4):
            o = zz[:, off(dd, ci, buf):off(dd, ci, buf) + 256]
            bsl = biasK_sb[:, (dd * 4 + ci) * 128:(dd * 4 + ci + 1) * 128]
            nc.tensor.matmul(out=o, lhsT=bsl,
                             rhs=ones16_sb[:, 0:256], start=True, stop=False,
                             skip_group_check=True)
            for (wT, rhs) in xparts[dd]:
                nc.tensor.matmul(out=o, lhsT=wT[:, ci * 128:(ci + 1) * 128],
                                 rhs=rhs[:, 256 * blk:256 * (blk + 1)],
                                 start=False, stop=False,
                                 skip_group_check=True)

    bulk(0, 0)
    bulk(1, NBLKS - 1)

    cF = wk.tile([128, 16], f32, tag="cF")
    cB = wk.tile([128, 16], f32, tag="cB")
    cs = (cF, cB)
    whs = (wh_f, wh_b)
    hs = (hF, hB)

    for s in range(T_):
        if s % 16 == 0 and s > 0:
            k = s // 16
            if k < NBLKS:
                bulk(0, k)
                bulk(1, NBLKS - 1 - k)
        tf, tb_t = s, T_ - 1 - s
        for dd, t in ((0, tf), (1, tb_t)):
            if s > 0:
                tp = t - 1 if dd == 0 else t + 1
                buf, tbi = (t // 16) % 2, t % 16
                wh = whs[dd]
                for ci in range(4):
                    o = zz[:, off(dd, ci, buf) + tbi * 16:
                           off(dd, ci, buf) + tbi * 16 + 16]
                    nc.tensor.matmul(out=o,
                                     lhsT=wh[:, ci * 128:(ci + 1) * 128],
                                     rhs=hs[dd][:, 16 * tp:16 * tp + 16],
                                     start=False, stop=True,
                                     skip_group_check=True)

        # fwd and bwd get fully separate ACT/DVE chains so the scheduler can
        # pipeline one direction's gates against the other's matmuls.
        for dd, t in ((0, tf), (1, tb_t)):
            buf, tbi = (t // 16) % 2, t % 16
            c_d = cs[dd]
            sig = wk.tile([128, 48], f32, tag=f"sig{dd}",
                          name=f"sig{dd}_{s}")
            nc.scalar.activation(out=sig[:], in_=zz6[:, dd, 0:3, buf, tbi],
                                 func=AF.Sigmoid)
            tg = wk.tile([128, 16], f32, tag=f"tg{dd}", name=f"tg{dd}_{s}")
            nc.scalar.activation(out=tg[:], in_=zz6[:, dd, 3, buf, tbi],
                                 func=AF.Tanh)
            if s == 0:
                nc.vector.tensor_mul(out=c_d[:], in0=sig[:, 0:16], in1=tg[:])
            else:
                t1 = wk.tile([128, 16], f32, tag=f"t1{dd}",
                             name=f"t1{dd}_{s}")
                nc.vector.tensor_mul(out=t1[:], in0=sig[:, 0:16], in1=tg[:])
                t2 = wk.tile([128, 16], f32, tag=f"t2{dd}",
                             name=f"t2{dd}_{s}")
                nc.vector.tensor_mul(out=t2[:], in0=sig[:, 16:32],
                                     in1=c_d[:])
                nc.vector.tensor_add(out=c_d[:], in0=t1[:], in1=t2[:])
            tct = wk.tile([128, 16], f32, tag=f"tct{dd}",
                          name=f"tct{dd}_{s}")
            nc.scalar.activation(out=tct[:], in_=c_d[:], func=AF.Tanh)
            nc.vector.tensor_mul(out=hs[dd][:, 16 * t:16 * t + 16],
                                 in0=sig[:, 32:48], in1=tct[:])
            if hs32 is not None and hs32[dd] is not None:
                nc.vector.tensor_mul(out=hs32[dd][:, 16 * t:16 * t + 16],
                                     in0=sig[:, 32:48], in1=tct[:])


def _emit(nc, tc, d, out_ids, T_, TOK, NBLK):
    # bisection aid: stop after a given stage (embed, l0, l1, proj, vit, full)
    stage = os.environ.get("KERNEL_STAGE", "full")
    order = ["embed", "l0", "l1", "proj", "vit", "full"]
    lvl = order.index(stage)
    with tc.tile_pool(name="gc", bufs=1) as gc:
        rep16_sb = gc.tile_from(d["rep16"][:])
        band_sb = [gc.tile_from(d[f"band{q}"][:], name=f"band{q}sb")
                   for q in range(8)]
        transP_sb = gc.tile_from(d["transP"][:])
        M8_sb = gc.tile_from(d["M8"][:])
        iotaI_sb = gc.tile_from(d["iotaI"][:])
        startRow_sb = gc.tile_from(d["startRow"][:])
        endRow_sb = gc.tile_from(d["endRow"][:])
        ones1_sb = gc.tile_from(d["ones1"][:])
        iotaF_sb = gc.tile_from(d["iotaF"][:])
        iotaJP_sb = gc.tile_from(d["iotaJP"][:])
        ident_sb = gc.tile_from(d["ident16"][:])
        ids_sb = gc.tile_from(d["ids_p"][:])
        woutA_sb = gc.tile_from(d["woutA"][:])
        woutB_sb = gc.tile_from(d["woutB"][:])
        ones16_sb = gc.tile_from(d["ones16"][:])
        bo1_sb = gc.tile_from(d["bo1"][:])

        outT = gc.tile([BL, T_], f32)
        em2 = gc.tile([128, NBLK * K], f32)
        histAll = gc.tile([128, 4 * T_], f32)
        out_sb = gc.tile([BL, T_], i32)

        transP3 = transP_sb.rearrange("p (g i) -> p g i", g=4)
        iotaI3 = iotaI_sb.rearrange("p (g i) -> p g i", g=4)
        M83 = M8_sb.rearrange("p (g jl) -> p g jl", g=4)

        with tc.tile_pool(name="h1p", bufs=1) as h1p:
            h1F = h1p.tile([128, TOK], f16)
            h1B = h1p.tile([128, TOK], f16)

            # ---------------- embedding gather + layer 0 ----------------
            with tc.tile_pool(name="px", bufs=1) as px, \
                 tc.tile_pool(name="wk0", bufs=3) as wk0:
                xT = px.tile([128, TOK], f16)
                w0 = {dd: (px.tile_from(d[f"wx0{dd}"][:], name=f"wx0{dd}sb"),
                           px.tile_from(d[f"wh0{dd}"][:], name=f"wh0{dd}sb"))
                      for dd in (0, 1)}
                biasK0_sb = px.tile_from(d["biasK0"][:])

                with tc.tile_pool(name="ge", bufs=4) as ge, \
                     tc.tile_pool(name="pe", bufs=2, space="PSUM") as pe:
                    # gather order: both ends first so fwd/bwd start early
                    order = []
                    for k in range((NBLK + 1) // 2):
                        order.append(k)
                        if NBLK - 1 - k != k:
                            order.append(NBLK - 1 - k)
                    for n, k in enumerate(order):
                        g_t = ge.tile([128, 128], f16, tag="g")
                        nc.gpsimd.indirect_dma_start(
                            out=g_t[:], out_offset=None, in_=d["embt"][:],
                            in_offset=IndirectOffsetOnAxis(
                                ap=ids_sb[:, k:k + 1], axis=0))
                        tp = pe.tile([128, 128], f16, tag="tp")
                        nc.tensor.transpose(tp[:], g_t[:], ident_sb[:])
                        dst = xT[:, 128 * k:128 * (k + 1)]
                        if n % 2 == 0:
                            nc.vector.tensor_copy(out=dst, in_=tp[:])
                        else:
                            nc.scalar.copy(out=dst, in_=tp[:])

                if lvl >= 1:
                    with tc.tile_pool(name="zp0", bufs=1,
                                      space="PSUM") as zp0:
                        zz0 = zp0.tile([128, 4096], f32)
                        xp = {dd: [(w0[dd][0], xT)] for dd in (0, 1)}
                        _lstm_layer(nc, wk0, zz0, T_, xp, w0[0][1], w0[1][1],
                                    biasK0_sb, ones16_sb, h1F, h1B)

            # ---------------- layer 1 ----------------
            with tc.tile_pool(name="pw1", bufs=1) as pw1, \
                 tc.tile_pool(name="ph2", bufs=1) as ph2:
                w1 = {dd: (pw1.tile_from(d[f"wxA1{dd}"][:], name=f"wxA1{dd}sb"),
                           pw1.tile_from(d[f"wxB1{dd}"][:], name=f"wxB1{dd}sb"),
                           pw1.tile_from(d[f"wh1{dd}"][:], name=f"wh1{dd}sb"))
                      for dd in (0, 1)}
                biasK1_sb = pw1.tile_from(d["biasK1"][:])
                h2F = ph2.tile([128, TOK], f16)
                h2F32 = ph2.tile([128, TOK], f32)
                h2B = ph2.tile([128, TOK], f16)
                h2B32 = ph2.tile([128, TOK], f32)

                if lvl >= 2:
                    with tc.tile_pool(name="zp1", bufs=1,
                                      space="PSUM") as zp1, \
                         tc.tile_pool(name="wk1", bufs=3) as wk1:
                        zz1 = zp1.tile([128, 4096], f32)
                        xp = {dd: [(w1[dd][0], h1F), (w1[dd][1], h1B)]
                              for dd in (0, 1)}
                        _lstm_layer(nc, wk1, zz1, T_, xp, w1[0][2], w1[1][2],
                                    biasK1_sb, ones16_sb, h2F, h2B,
                                    hs32=(h2F32, h2B32))

                # ---------------- emission projection ----------------
                # em2[(t%8)*16+b, (t//8)*32+j]  (j = raw tag index)
                with tc.tile_pool(name="pj", bufs=2, space="PSUM") as pj:
                    for k in range(NBLK if lvl >= 3 else 0):
                        ep = pj.tile([128, K], f32, tag="ep")
                        nc.tensor.matmul(out=ep[:], lhsT=ones1_sb[:],
                                         rhs=bo1_sb[:], start=True,
                                         stop=False, skip_group_check=True)
                        nc.tensor.matmul(out=ep[:],
                                         lhsT=h2F32[:, 128 * k:128 * (k + 1)],
                                         rhs=woutA_sb[:], start=False,
                                         stop=False, skip_group_check=True)
                        nc.tensor.matmul(out=ep[:],
                                         lhsT=h2B32[:, 128 * k:128 * (k + 1)],
                                         rhs=woutB_sb[:], start=False,
                                         stop=True, skip_group_check=True)
                        dst = em2[:, K * k:K * (k + 1)]
                        if k % 2 == 0:
                            nc.vector.tensor_copy(out=dst, in_=ep[:])
                        else:
                            nc.scalar.copy(out=dst, in_=ep[:])

        # ---------------- Viterbi forward ----------------
        # State ms [128=(jl,b), 4=g] holds score[b, g*8+jl].  Each step:
        #   Rt = broadcast_g(ms) * M8            (DVE, [128,32])
        #   sr = band[tl] @ em_blk + rep16 @ Rt  (PE -> [128,32] replicated)
        #   cand = sr + transP; ms = max_i cand; hist = argmax_i cand
        hA3 = histAll.rearrange("p (g t) -> p g t", t=T_)
        with tc.tile_pool(name="srp", bufs=2, space="PSUM") as srp, \
             tc.tile_pool(name="vt", bufs=3) as vt:
            nc.vector.memset(hA3[:, :, 0], 0.0)
            prev_ms = None
            for s in range(1, (T_ + 1) if lvl >= 4 else 0):
                sr = srp.tile([128, K], f32, tag="sr")
                tl = (s - 1) % 8
                blk = (s - 1) // 8
                nc.tensor.matmul(out=sr[:],
                                 lhsT=band_sb[tl][:],
                                 rhs=em2[:, K * blk:K * (blk + 1)],
                                 start=True, stop=False,
                                 skip_group_check=True)
                if s == 1:
                    nc.tensor.matmul(out=sr[:], lhsT=ones1_sb[:],
                                     rhs=startRow_sb[:], start=False,
                                     stop=True, skip_group_check=True)
                else:
                    Rt = vt.tile([128, K], f32, tag="Rt", name=f"Rt_{s}")
                    Rt3 = Rt.rearrange("p (g jl) -> p g jl", g=4)
                    msb = prev_ms.rearrange("p (g o) -> p g o", o=1) \
                                 .to_broadcast([128, 4, 8])
                    nc.vector.tensor_mul(out=Rt3, in0=msb, in1=M83)
                    nc.tensor.matmul(out=sr[:], lhsT=rep16_sb[:],
                                     rhs=Rt[:], start=False,
                                     stop=(s != T_), skip_group_check=True)
                if s == T_:
                    # fold end transition scores into the final step
                    nc.tensor.matmul(out=sr[:], lhsT=ones1_sb[:],
                                     rhs=endRow_sb[:], start=False,
                                     stop=True, skip_group_check=True)
                    sf = vt.tile([BL, K], f32, tag="sf")
                    nc.vector.tensor_copy(out=sf[:], in_=sr[0:16, :])
                    mfin = vt.tile([BL, 1], f32, tag="mfin")
                    nc.vector.reduce_max(out=mfin[:], in_=sf[:], axis=AX.X)
                    eqf = vt.tile([BL, K], f32, tag="eqf")
                    nc.vector.tensor_tensor(
                        out=eqf[:], in0=sf[:],
                        in1=mfin[:].to_broadcast([BL, K]), op=ALU.is_equal)
                    eif = vt.tile([BL, K], f32, tag="eif")
                    nc.vector.tensor_mul(out=eif[:], in0=eqf[:],
                                         in1=iotaF_sb[:])
                    nc.vector.reduce_max(out=outT[:, T_ - 1:T_], in_=eif[:],
                                         axis=AX.X)
                    break

                cand = vt.tile([128, 128], f32, tag="cand")
                cand3 = cand.rearrange("p (g i) -> p g i", g=4)
                srb = sr[:].rearrange("p (o i) -> p o i", o=1) \
                           .to_broadcast([128, 4, K])
                nc.vector.tensor_add(out=cand3, in0=srb, in1=transP3)
                ms = vt.tile([128, 4], f32, tag="ms", name=f"ms_{s}")
                nc.vector.reduce_max(out=ms[:], in_=cand3, axis=AX.X)
                eqv = vt.tile([128, 128], f32, tag="eqv")
                eq3 = eqv.rearrange("p (g i) -> p g i", g=4)
                msb2 = ms[:].rearrange("p (g o) -> p g o", o=1) \
                            .to_broadcast([128, 4, K])
                nc.vector.tensor_tensor(out=eq3, in0=cand3, in1=msb2,
                                        op=ALU.is_equal)
                eiv = vt.tile([128, 128], f32, tag="eiv")
                ei3 = eiv.rearrange("p (g i) -> p g i", g=4)
                nc.vector.tensor_mul(out=ei3, in0=eq3, in1=iotaI3)
                nc.vector.reduce_max(out=hA3[:, :, s], in_=ei3, axis=AX.X)
                prev_ms = ms

        # ---------------- backtrace ----------------
        with tc.tile_pool(name="pbt", bufs=1) as pbt, \
             tc.tile_pool(name="bt", bufs=2) as bt:
            if lvl >= 5:
                histAllB = pbt.tile([128, 4 * T_], bf16)
                nc.vector.tensor_copy(out=histAllB[:], in_=histAll[:])
                histB = pbt.tile([BL, 32 * T_], bf16)
                hB4 = histB.rearrange("p (jl g t) -> p jl g t", jl=8, g=4)
                for jl in range(8):
                    src = histAllB[16 * jl:16 * (jl + 1), :] \
                        .rearrange("p (g t) -> p g t", t=T_)
                    nc.sync.dma_start(out=hB4[:, jl], in_=src)

                for s in range(T_ - 2, -1, -1):
                    oh = bt.tile([BL, K], bf16, tag="oh")
                    nc.vector.tensor_scalar(out=oh[:], in0=iotaJP_sb[:],
                                            scalar1=outT[:, s + 1:s + 2],
                                            scalar2=None, op0=ALU.is_equal)
                    oh3 = oh.rearrange("p (jl g) -> p jl g", jl=8)
                    scr = bt.tile([BL, K], bf16, tag="scr")
                    scr3 = scr.rearrange("p (jl g) -> p jl g", jl=8)
                    nc.vector.tensor_mul(out=scr3, in0=oh3,
                                         in1=hB4[:, :, :, s + 1])
                    nc.vector.reduce_max(out=outT[:, s:s + 1], in_=scr[:],
                                         axis=AX.X)

                nc.vector.tensor_copy(out=out_sb[:], in_=outT[:])
            else:
                nc.vector.memset(out_sb[:], 0)
            nc.sync.dma_start(out=out_ids[:], in_=out_sb[:])


def _run(inputs_np, consts, T_):
    global LAST_RESULTS
    nc = _build_program(T_)
    in_maps = []
    for core in range(NCORES):
        m = dict(consts)
        m["ids_p"] = _ids_for_core(inputs_np, core, T_)
        in_maps.append(m)
    trace = bool(int(os.environ.get("KERNEL_TRACE", "0")))
    res = bass_utils.run_bass_kernel_spmd(
        nc, in_maps, core_ids=list(range(NCORES)), trace=trace)
    LAST_RESULTS = res
    return np.concatenate([r["out_ids"] for r in res.results], axis=0)


def kernel(inputs, tags, emb, w_ih_l0, w_hh_l0, b_l0,
           w_ih_l1, w_hh_l1, b_l1, W_out, b_out,
           start_t, end_t, trans, _T=TFULL):
    del tags  # unused at decode time
    inputs_np = np.ascontiguousarray(np.asarray(inputs, dtype=np.int32))
    consts = _host_consts(emb, w_ih_l0, w_hh_l0, b_l0, w_ih_l1, w_hh_l1,
                          b_l1, W_out, b_out, start_t, end_t, trans)
    return _run(inputs_np, consts, _T)


# revision 16
# speedup vs baseline: 3.3676x; 1.0341x over previous
"""BiLSTM-CRF decode kernel for Trainium2 (8 NeuronCores, batch-sharded).

Model: embedding lookup -> 2-layer BiLSTM (H=128/dir) -> linear -> CRF Viterbi.
Output: [B, T] int32 best-path tags.

Sharding: data-parallel over batch, B=128 -> 16 rows per core. Feature-major
layout ([feature partitions, batch free]) so the serial time recurrences run
full-width engine ops.

Perf structure (v2):
- The input-to-gates GEMMs (x @ Wih) are hoisted out of the recurrence into
  bulk N=256 matmuls that pre-accumulate zx+bias into PSUM half-banks; the
  in-loop recurrence only adds Whh @ h into the same PSUM cells. The gate
  nonlinearities read PSUM directly.
- Precision split (fp16 matmuls are 1-pass on the PE, fp32 are 2-pass):
  everything runs fp16 except the l1-backward g-gate recurrence matmul and
  the emission projection, which stay fp32 (h2 kept in fp16+fp32 copies).
  The g-gate dominates error injection (dc/dz_g ~ 0.5 vs ~0.06 for i/f/o):
  fp16 i/f/o in l1-bwd adds zero tag flips, fp16 g there adds ~36.
  Measured 24/65536 flips (rel err 1.33e-2) vs the 2e-2 gate.
- Viterbi forward uses a replicated-score formulation: one masked DVE mul +
  one rep16 matmul replaces the 8 per-step band broadcasts.
"""

import os
import numpy as np

import concourse.bass as bass
import concourse.bacc as bacc
import concourse.tile as tile
import concourse.mybir as mybir
from concourse.bass import IndirectOffsetOnAxis
from concourse import bass_utils

B, TFULL, V, D, H, K = 128, 512, 50000, 128, 128, 32
NCORES = 8
BL = B // NCORES  # 16 batch rows per core

f32 = mybir.dt.float32
f16 = mybir.dt.float16
bf16 = mybir.dt.bfloat16
i32 = mybir.dt.int32
AF = mybir.ActivationFunctionType
ALU = mybir.AluOpType
AX = mybir.AxisListType

# torch gate order is [i, f, g, o]; we use [i, f, o, g] so the sigmoid gates
# (i, f, o) are contiguous and tanh(g) is the last chunk.
_PERM = np.r_[0:H, H:2 * H, 3 * H:4 * H, 2 * H:3 * H]

LAST_RESULTS = None  # BassKernelResults of the most recent run (for test.py)


def _f(x):
    return np.ascontiguousarray(np.asarray(x, dtype=np.float32))


def _h(x):
    return np.ascontiguousarray(np.asarray(x, dtype=np.float16))


def _host_consts(emb, w_ih_l0, w_hh_l0, b_l0, w_ih_l1, w_hh_l1, b_l1,
                 W_out, b_out, start_t, end_t, trans):
    """Build all per-core-identical device input arrays."""
    c = {}
    c["embt"] = _h(emb)

    for d in (0, 1):
        c[f"wx0{d}"] = _h(np.asarray(w_ih_l0)[d][_PERM].T)      # [128, 512]
        c[f"wh0{d}"] = _h(np.asarray(w_hh_l0)[d][_PERM].T)      # [128, 512]
        w1 = np.asarray(w_ih_l1)[d][_PERM]                       # [512, 256]
        c[f"wxA1{d}"] = _h(w1[:, :H].T)                          # [128, 512]
        c[f"wxB1{d}"] = _h(w1[:, H:].T)                          # [128, 512]
        c[f"wh1{d}"] = _h(np.asarray(w_hh_l1)[d][_PERM].T)       # [128, 512]

    for l, bl in ((0, b_l0), (1, b_l1)):
        bk = np.zeros((1, 1024), dtype=np.float16)
        for d in (0, 1):
            bperm = np.asarray(bl)[d][_PERM]
            bk[0, d * 512:(d + 1) * 512] = bperm
        c[f"biasK{l}"] = np.ascontiguousarray(bk)
    c["ones16"] = np.ones((1, 512), dtype=np.float16)

    # Emission projection (b_out folded into trans/start below).
    WoT = _f(np.asarray(W_out).T)                                # [256, 32]
    c["woutA"] = _f(WoT[:H])
    c["woutB"] = _f(WoT[H:])

    km = np.arange(128)
    rep_full = (km[:, None] % 16 == km[None, :] % 16).astype(np.float32)
    c["rep16"] = _f(rep_full)
    for q in range(8):
        bm = rep_full.copy()
        bm[(km // 16) != q, :] = 0.0
        c[f"band{q}"] = _f(bm)

    # Viterbi, tag layout: score column i = tag i; per-partition-block jl
    # owns tags j = g*8 + jl (g in 0..3) stored as ms[:, g].
    trans_p = _f(trans)                                          # [32, 32]
    jlv = np.arange(128) // 16                                   # [128]
    gv = np.arange(4)
    # transP[(jl,b), g*32 + i] = trans_p[i, g*8 + jl]
    tp = np.zeros((128, 128), dtype=np.float32)
    for p in range(128):
        for g in range(4):
            tp[p, g * 32:(g + 1) * 32] = trans_p[:, g * 8 + jlv[p]]
    c["transP"] = tp
    # M8[(jl2,b), (g,jl)] = (jl2 == jl)
    m8 = np.zeros((128, 32), dtype=np.float32)
    for p in range(128):
        for g in range(4):
            for jl in range(8):
                m8[p, g * 8 + jl] = 1.0 if (p // 16) == jl else 0.0
    c["M8"] = m8
    c["iotaI"] = _f(np.tile(np.arange(32, dtype=np.float32), (128, 4)))
    c["startRow"] = _f(np.asarray(start_t))[None, :]
    c["bo1"] = _f(np.asarray(b_out))[None, :]
    c["endRow"] = _f(np.asarray(end_t))[None, :]
    c["ones1"] = np.ones((1, 128), dtype=np.float32)
    c["iotaF"] = _f(np.tile(np.arange(32, dtype=np.float32), (BL, 1)))

    jp = np.empty(32, dtype=np.float32)
    for jl in range(8):
        for g in range(4):
            jp[jl * 4 + g] = g * 8 + jl
    c["iotaJP"] = _f(np.tile(jp, (BL, 1)))                       # [16, 32]

    c["ident16"] = np.eye(128, dtype=np.float16)
    return c


def _ids_for_core(inputs_np, core, T_):
    ids_c = inputs_np[core * BL:(core + 1) * BL, :T_]            # [16, T]
    flat = np.ascontiguousarray(ids_c.T).reshape(-1)             # t-major
    nblk = (BL * T_) // 128
    return np.ascontiguousarray(flat.reshape(nblk, 128).T.astype(np.int32))


def _build_program(T_):
    """Build the full single-core Bass program (identical across cores)."""
    TOK = BL * T_
    NBLK = TOK // 128

    nc = bacc.Bacc()
    d = {}

    def din(name, shape, dtype=f32):
        d[name] = nc.dram_tensor(name, list(shape), dtype, kind="ExternalInput")
        return d[name]

    din("ids_p", [128, NBLK], i32)
    din("embt", [V, D], f16)
    for dd in (0, 1):
        din(f"wx0{dd}", [128, 512], f16)
        din(f"wh0{dd}", [128, 512], f16)
        din(f"wxA1{dd}", [128, 512], f16)
        din(f"wxB1{dd}", [128, 512], f16)
        din(f"wh1{dd}", [128, 512], f16)
    din("biasK0", [1, 1024], f16)
    din("biasK1", [1, 1024], f16)
    din("ones16", [1, 512], f16)
    din("woutA", [128, K])
    din("woutB", [128, K])
    din("rep16", [128, 128])
    for q in range(8):
        din(f"band{q}", [128, 128])
    din("transP", [128, 128])
    din("M8", [128, K])
    din("iotaI", [128, 128])
    din("startRow", [1, K])
    din("bo1", [1, K])
    din("endRow", [1, K])
    din("ones1", [1, 128])
    din("iotaF", [BL, K])
    din("iotaJP", [BL, K])
    din("ident16", [128, 128], f16)
    out_ids = nc.dram_tensor("out_ids", [BL, T_], i32, kind="ExternalOutput")

    with tile.TileContext(nc) as tc:
        _emit(nc, tc, d, out_ids, T_, TOK, NBLK)
    nc.compile()
    return nc


def _lstm_layer(nc, wk, zz, T_, xparts, wh_f, wh_b, biasK_sb, ones16_sb,
                hF, hB, hs32=None):
    """One BiLSTM layer; fwd and bwd directions interleaved per step.

    zz: PSUM tile [128, 4096] = 8 banks; bank (dd*4+ci) holds
    [buf(2), tb(16), b(16)] columns. Bulk zx+bias matmuls pre-fill 16-step
    half-banks; the in-loop Whh matmuls accumulate into single-step cells.
    xparts: per-dir list of (wxT [128,512] f16 sbuf, rhs [128, TOK] f16 sbuf).
    hF/hB: [128, TOK] f16 output buffers (also the recurrent matmul input).
    """
    NBLKS = T_ // 16
    zz6 = zz.rearrange("p (dd ci buf tb b) -> p dd ci buf tb b",
                       dd=2, ci=4, buf=2, tb=16)

    def off(dd, ci, buf):
        return ((dd * 4 + ci) * 2 + buf) * 256

    def bulk(dd, blk):
        buf = blk % 2
        for ci in range(4):
            o = zz[:, off(dd, ci, buf):off(dd, ci, buf) + 256]
            bsl = biasK_sb[:, (dd * 4 + ci) * 128:(dd * 4 + ci + 1) * 128]
            nc.tensor.matmul(out=o, lhsT=bsl,
                             rhs=ones16_sb[:, 0:256], start=True, stop=False,
                             skip_group_check=True)
            for (wT, rhs) in xparts[dd]:
                nc.tensor.matmul(out=o, lhsT=wT[:, ci * 128:(ci + 1) * 128],
                                 rhs=rhs[:, 256 * blk:256 * (blk + 1)],
                                 start=False, stop=False,
                                 skip_group_check=True)

    bulk(0, 0)
    bulk(1, NBLKS - 1)

    cF = wk.tile([128, 16], f32, tag="cF")
    cB = wk.tile([128, 16], f32, tag="cB")
    cs = (cF, cB)
    whs = (wh_f, wh_b)
    hs = (hF, hB)

    for s in range(T_):
        if s % 16 == 0 and s > 0:
            k = s // 16
            if k < NBLKS:
                bulk(0, k)
                bulk(1, NBLKS - 1 - k)
        tf, tb_t = s, T_ - 1 - s
        for dd, t in ((0, tf), (1, tb_t)):
            if s > 0:
                tp = t - 1 if dd == 0 else t + 1
                buf, tbi = (t // 16) % 2, t % 16
                wh = whs[dd]
                for ci in range(4):
                    o = zz[:, off(dd, ci, buf) + tbi * 16:
                           off(dd, ci, buf) + tbi * 16 + 16]
                    nc.tensor.matmul(out=o,
                                     lhsT=wh[:, ci * 128:(ci + 1) * 128],
                                     rhs=hs[dd][:, 16 * tp:16 * tp + 16],
                                     start=False, stop=True,
                                     skip_group_check=True)

        # fwd and bwd get fully separate ACT/DVE chains so the scheduler can
        # pipeline one direction's gates against the other's matmuls.
        for dd, t in ((0, tf), (1, tb_t)):
            buf, tbi = (t // 16) % 2, t % 16
            c_d = cs[dd]
            sig = wk.tile([128, 48], f32, tag=f"sig{dd}",
                          name=f"sig{dd}_{s}")
            nc.scalar.activation(out=sig[:], in_=zz6[:, dd, 0:3, buf, tbi],
                                 func=AF.Sigmoid)
            tg = wk.tile([128, 16], f32, tag=f"tg{dd}", name=f"tg{dd}_{s}")
            nc.scalar.activation(out=tg[:], in_=zz6[:, dd, 3, buf, tbi],
                                 func=AF.Tanh)
            if s == 0:
                nc.vector.tensor_mul(out=c_d[:], in0=sig[:, 0:16], in1=tg[:])
            else:
                t1 = wk.tile([128, 16], f32, tag=f"t1{dd}",
                             name=f"t1{dd}_{s}")
                nc.vector.tensor_mul(out=t1[:], in0=sig[:, 0:16], in1=tg[:])
                t2 = wk.tile([128, 16], f32, tag=f"t2{dd}",
                             name=f"t2{dd}_{s}")
                nc.vector.tensor_mul(out=t2[:], in0=sig[:, 16:32],
                                     in1=c_d[:])
                nc.vector.tensor_add(out=c_d[:], in0=t1[:], in1=t2[:])
            tct = wk.tile([128, 16], f32, tag=f"tct{dd}",
                          name=f"tct{dd}_{s}")
            nc.scalar.activation(out=tct[:], in_=c_d[:], func=AF.Tanh)
            nc.vector.tensor_mul(out=hs[dd][:, 16 * t:16 * t + 16],
                                 in0=sig[:, 32:48], in1=tct[:])
            if hs32 is not None and hs32[dd] is not None:
                nc.vector.tensor_mul(out=hs32[dd][:, 16 * t:16 * t + 16],
                                     in0=sig[:, 32:48], in1=tct[:])


def _emit(nc, tc, d, out_ids, T_, TOK, NBLK):
    # bisection aid: stop after a given stage (embed, l0, l1, proj, vit, full)
    stage = os.environ.get("KERNEL_STAGE", "full")
    order = ["embed", "l0", "l1", "proj", "vit", "full"]
    lvl = order.index(stage)
    with tc.tile_pool(name="gc", bufs=1) as gc:
        rep16_sb = gc.tile_from(d["rep16"][:])
        band_sb = [gc.tile_from(d[f"band{q}"][:], name=f"band{q}sb")
                   for q in range(8)]
        transP_sb = gc.tile_from(d["transP"][:])
        M8_sb = gc.tile_from(d["M8"][:])
        iotaI_sb = gc.tile_from(d["iotaI"][:])
        startRow_sb = gc.tile_from(d["startRow"][:])
        endRow_sb = gc.tile_from(d["endRow"][:])
        ones1_sb = gc.tile_from(d["ones1"][:])
        iotaF_sb = gc.tile_from(d["iotaF"][:])
        iotaJP_sb = gc.tile_from(d["iotaJP"][:])
        ident_sb = gc.tile_from(d["ident16"][:])
        ids_sb = gc.tile_from(d["ids_p"][:])
        woutA_sb = gc.tile_from(d["woutA"][:])
        woutB_sb = gc.tile_from(d["woutB"][:])
        ones16_sb = gc.tile_from(d["ones16"][:])
        bo1_sb = gc.tile_from(d["bo1"][:])

        outT = gc.tile([BL, T_], f32)
        em2 = gc.tile([128, NBLK * K], f32)
        histAll = gc.tile([128, 4 * T_], f32)
        out_sb = gc.tile([BL, T_], i32)

        transP3 = transP_sb.rearrange("p (g i) -> p g i", g=4)
        iotaI3 = iotaI_sb.rearrange("p (g i) -> p g i", g=4)
        M83 = M8_sb.rearrange("p (g jl) -> p g jl", g=4)

        with tc.tile_pool(name="h1p", bufs=1) as h1p:
            h1F = h1p.tile([128, TOK], f16)
            h1B = h1p.tile([128, TOK], f16)

            # ---------------- embedding gather + layer 0 ----------------
            with tc.tile_pool(name="px", bufs=1) as px, \
                 tc.tile_pool(name="wk0", bufs=4) as wk0:
                xT = px.tile([128, TOK], f16)
                w0 = {dd: (px.tile_from(d[f"wx0{dd}"][:], name=f"wx0{dd}sb"),
                           px.tile_from(d[f"wh0{dd}"][:], name=f"wh0{dd}sb"))
                      for dd in (0, 1)}
                biasK0_sb = px.tile_from(d["biasK0"][:])

                with tc.tile_pool(name="ge", bufs=6) as ge, \
                     tc.tile_pool(name="pe", bufs=3, space="PSUM") as pe:
                    # gather order: both ends first so fwd/bwd start early
                    order = []
                    for k in range((NBLK + 1) // 2):
                        order.append(k)
                        if NBLK - 1 - k != k:
                            order.append(NBLK - 1 - k)
                    for n, k in enumerate(order):
                        g_t = ge.tile([128, 128], f16, tag="g")
                        nc.gpsimd.indirect_dma_start(
                            out=g_t[:], out_offset=None, in_=d["embt"][:],
                            in_offset=IndirectOffsetOnAxis(
                                ap=ids_sb[:, k:k + 1], axis=0))
                        tp = pe.tile([128, 128], f16, tag="tp")
                        nc.tensor.transpose(tp[:], g_t[:], ident_sb[:])
                        dst = xT[:, 128 * k:128 * (k + 1)]
                        if n % 2 == 0:
                            nc.vector.tensor_copy(out=dst, in_=tp[:])
                        else:
                            nc.scalar.copy(out=dst, in_=tp[:])

                if lvl >= 1:
                    with tc.tile_pool(name="zp0", bufs=1,
                                      space="PSUM") as zp0:
                        zz0 = zp0.tile([128, 4096], f32)
                        xp = {dd: [(w0[dd][0], xT)] for dd in (0, 1)}
                        _lstm_layer(nc, wk0, zz0, T_, xp, w0[0][1], w0[1][1],
                                    biasK0_sb, ones16_sb, h1F, h1B)

            # ---------------- layer 1 ----------------
            with tc.tile_pool(name="pw1", bufs=1) as pw1, \
                 tc.tile_pool(name="ph2", bufs=1) as ph2:
                w1 = {dd: (pw1.tile_from(d[f"wxA1{dd}"][:], name=f"wxA1{dd}sb"),
                           pw1.tile_from(d[f"wxB1{dd}"][:], name=f"wxB1{dd}sb"),
                           pw1.tile_from(d[f"wh1{dd}"][:], name=f"wh1{dd}sb"))
                      for dd in (0, 1)}
                biasK1_sb = pw1.tile_from(d["biasK1"][:])
                h2F = ph2.tile([128, TOK], f16)
                h2F32 = ph2.tile([128, TOK], f32)
                h2B = ph2.tile([128, TOK], f16)
                h2B32 = ph2.tile([128, TOK], f32)

                if lvl >= 2:
                    with tc.tile_pool(name="zp1", bufs=1,
                                      space="PSUM") as zp1, \
                         tc.tile_pool(name="wk1", bufs=4) as wk1:
                        zz1 = zp1.tile([128, 4096], f32)
                        xp = {dd: [(w1[dd][0], h1F), (w1[dd][1], h1B)]
                              for dd in (0, 1)}
                        _lstm_layer(nc, wk1, zz1, T_, xp, w1[0][2], w1[1][2],
                                    biasK1_sb, ones16_sb, h2F, h2B,
                                    hs32=(h2F32, h2B32))

                # ---------------- emission projection ----------------
                # em2[(t%8)*16+b, (t//8)*32+j]  (j = raw tag index)
                with tc.tile_pool(name="pj", bufs=2, space="PSUM") as pj:
                    for k in range(NBLK if lvl >= 3 else 0):
                        ep = pj.tile([128, K], f32, tag="ep")
                        nc.tensor.matmul(out=ep[:], lhsT=ones1_sb[:],
                                         rhs=bo1_sb[:], start=True,
                                         stop=False, skip_group_check=True)
                        nc.tensor.matmul(out=ep[:],
                                         lhsT=h2F32[:, 128 * k:128 * (k + 1)],
                                         rhs=woutA_sb[:], start=False,
                                         stop=False, skip_group_check=True)
                        nc.tensor.matmul(out=ep[:],
                                         lhsT=h2B32[:, 128 * k:128 * (k + 1)],
                                         rhs=woutB_sb[:], start=False,
                                         stop=True, skip_group_check=True)
                        dst = em2[:, K * k:K * (k + 1)]
                        if k % 2 == 0:
                            nc.vector.tensor_copy(out=dst, in_=ep[:])
                        else:
                            nc.scalar.copy(out=dst, in_=ep[:])

        # ---------------- Viterbi forward ----------------
        # State ms [128=(jl,b), 4=g] holds score[b, g*8+jl].  Each step:
        #   Rt = broadcast_g(ms) * M8            (DVE, [128,32])
        #   sr = band[tl] @ em_blk + rep16 @ Rt  (PE -> [128,32] replicated)
        #   cand = sr + transP; ms = max_i cand; hist = argmax_i cand
        hA3 = histAll.rearrange("p (g t) -> p g t", t=T_)
        with tc.tile_pool(name="srp", bufs=2, space="PSUM") as srp, \
             tc.tile_pool(name="vt", bufs=4) as vt:
            nc.vector.memset(hA3[:, :, 0], 0.0)
            prev_ms = None
            for s in range(1, (T_ + 1) if lvl >= 4 else 0):
                sr = srp.tile([128, K], f32, tag="sr")
                tl = (s - 1) % 8
                blk = (s - 1) // 8
                nc.tensor.matmul(out=sr[:],
                                 lhsT=band_sb[tl][:],
                                 rhs=em2[:, K * blk:K * (blk + 1)],
                                 start=True, stop=False,
                                 skip_group_check=True)
                if s == 1:
                    nc.tensor.matmul(out=sr[:], lhsT=ones1_sb[:],
                                     rhs=startRow_sb[:], start=False,
                                     stop=True, skip_group_check=True)
                else:
                    Rt = vt.tile([128, K], f32, tag="Rt", name=f"Rt_{s}")
                    Rt3 = Rt.rearrange("p (g jl) -> p g jl", g=4)
                    msb = prev_ms.rearrange("p (g o) -> p g o", o=1) \
                                 .to_broadcast([128, 4, 8])
                    nc.vector.tensor_mul(out=Rt3, in0=msb, in1=M83)
                    nc.tensor.matmul(out=sr[:], lhsT=rep16_sb[:],
                                     rhs=Rt[:], start=False,
                                     stop=(s != T_), skip_group_check=True)
                if s == T_:
                    # fold end transition scores into the final step
                    nc.tensor.matmul(out=sr[:], lhsT=ones1_sb[:],
                                     rhs=endRow_sb[:], start=False,
                                     stop=True, skip_group_check=True)
                    sf = vt.tile([BL, K], f32, tag="sf")
                    nc.vector.tensor_copy(out=sf[:], in_=sr[0:16, :])
                    mfin = vt.tile([BL, 1], f32, tag="mfin")
                    nc.vector.reduce_max(out=mfin[:], in_=sf[:], axis=AX.X)
                    eqf = vt.tile([BL, K], f32, tag="eqf")
                    nc.vector.tensor_tensor(
                        out=eqf[:], in0=sf[:],
                        in1=mfin[:].to_broadcast([BL, K]), op=ALU.is_equal)
                    eif = vt.tile([BL, K], f32, tag="eif")
                    nc.vector.tensor_mul(out=eif[:], in0=eqf[:],
                                         in1=iotaF_sb[:])
                    nc.vector.reduce_max(out=outT[:, T_ - 1:T_], in_=eif[:],
                                         axis=AX.X)
                    break

                cand = vt.tile([128, 128], f32, tag="cand")
                cand3 = cand.rearrange("p (g i) -> p g i", g=4)
                srb = sr[:].rearrange("p (o i) -> p o i", o=1) \
                           .to_broadcast([128, 4, K])
                nc.vector.tensor_add(out=cand3, in0=srb, in1=transP3)
                ms = vt.tile([128, 4], f32, tag="ms", name=f"ms_{s}")
                nc.vector.reduce_max(out=ms[:], in_=cand3, axis=AX.X)
                eqv = vt.tile([128, 128], f32, tag="eqv")
                eq3 = eqv.rearrange("p (g i) -> p g i", g=4)
                msb2 = ms[:].rearrange("p (g o) -> p g o", o=1) \
                            .to_broadcast([128, 4, K])
                nc.vector.tensor_tensor(out=eq3, in0=cand3, in1=msb2,
                                        op=ALU.is_equal)
                eiv = vt.tile([128, 128], f32, tag="eiv")
                ei3 = eiv.rearrange("p (g i) -> p g i", g=4)
                nc.vector.tensor_mul(out=ei3, in0=eq3, in1=iotaI3)
                nc.vector.reduce_max(out=hA3[:, :, s], in_=ei3, axis=AX.X)
                prev_ms = ms

        # ---------------- backtrace ----------------
        with tc.tile_pool(name="pbt", bufs=1) as pbt, \
             tc.tile_pool(name="bt", bufs=2) as bt:
            if lvl >= 5:
                histAllB = pbt.tile([128, 4 * T_], bf16)
                nc.vector.tensor_copy(out=histAllB[:], in_=histAll[:])
                histB = pbt.tile([BL, 32 * T_], bf16)
                hB4 = histB.rearrange("p (jl g t) -> p jl g t", jl=8, g=4)
                for jl in range(8):
                    src = histAllB[16 * jl:16 * (jl + 1), :] \
                        .rearrange("p (g t) -> p g t", t=T_)
                    nc.sync.dma_start(out=hB4[:, jl], in_=src)

                for s in range(T_ - 2, -1, -1):
                    oh = bt.tile([BL, K], bf16, tag="oh")
                    nc.vector.tensor_scalar(out=oh[:], in0=iotaJP_sb[:],
                                            scalar1=outT[:, s + 1:s + 2],
                                            scalar2=None, op0=ALU.is_equal)
                    oh3 = oh.rearrange("p (jl g) -> p jl g", jl=8)
                    scr = bt.tile([BL, K], bf16, tag="scr")
                    scr3 = scr.rearrange("p (jl g) -> p jl g", jl=8)
                    nc.vector.tensor_mul(out=scr3, in0=oh3,
                                         in1=hB4[:, :, :, s + 1])
                    nc.vector.reduce_max(out=outT[:, s:s + 1], in_=scr[:],
                                         axis=AX.X)

                nc.vector.tensor_copy(out=out_sb[:], in_=outT[:])
            else:
                nc.vector.memset(out_sb[:], 0)
            nc.sync.dma_start(out=out_ids[:], in_=out_sb[:])


def _run(inputs_np, consts, T_):
    global LAST_RESULTS
    nc = _build_program(T_)
    in_maps = []
    for core in range(NCORES):
        m = dict(consts)
        m["ids_p"] = _ids_for_core(inputs_np, core, T_)
        in_maps.append(m)
    trace = bool(int(os.environ.get("KERNEL_TRACE", "0")))
    res = bass_utils.run_bass_kernel_spmd(
        nc, in_maps, core_ids=list(range(NCORES)), trace=trace)
    LAST_RESULTS = res
    return np.concatenate([r["out_ids"] for r in res.results], axis=0)


def kernel(inputs, tags, emb, w_ih_l0, w_hh_l0, b_l0,
           w_ih_l1, w_hh_l1, b_l1, W_out, b_out,
           start_t, end_t, trans, _T=TFULL):
    del tags  # unused at decode time
    inputs_np = np.ascontiguousarray(np.asarray(inputs, dtype=np.int32))
    consts = _host_consts(emb, w_ih_l0, w_hh_l0, b_l0, w_ih_l1, w_hh_l1,
                          b_l1, W_out, b_out, start_t, end_t, trans)
    return _run(inputs_np, consts, _T)
